# revision 1
# baseline (speedup 1.0000x reference)
"""nn_Attention_30511447671564 — Trainium2 Bass kernel.

Head-mixing attention block. Shapes (hardcoded): B=64, T=64, C=4096,
H=64, hd=64, rank=1.  For every token (b,t): attention mixes the 64
heads (HxH scores, causal over head index).

    qkv = x @ W_atten^T + b_atten                  (B,T,3C)
    per-token: s[i,j] = q_i . k_j / 8  (i,j heads, causal j<=i)
               att = softmax_j(s);  y_i = sum_j att[i,j] v_j
    out = y @ W_proj^T + b_proj                    (B,T,C)

Distribution: pure data-parallel — 8 cores x 512 tokens, no collectives.

Per-core device pipeline (all matmuls bf16 with fp32 PSUM accumulation):
  1. QKV "Form 2": qkvT[o, t] = sum_c WaT[c, o] * xT[c, t]; bias (and the
     1/8 score scale, folded into k) applied on PSUM eviction.
  2. Rotation: evicted feature tiles [(h,d) x t] are DMA'd to DRAM bounce
     tensors Qrot/Krot [d, h, t] and Vrot [h, d, t] (contiguous runs).
  3. Attention in 128-token chunks: per token, 64x64 matmuls
     sT = K_t^T-style scores with heads contracted over d; exp (no max
     subtraction — scores are O(10), safe in fp32); causal mask applied
     multiplicatively after exp; softmax denominators via a ones-column
     matmul off the same stationary attT; normalization folded into the
     PSUM eviction scale. y is written (d-major) and DMA'd to Ydram.
  4. Projection "Form 2" from Ydram, bias on eviction, transposed output
     outT[c, t] written to DRAM; host un-transposes.
"""

import numpy as np
import ml_dtypes
from contextlib import ExitStack

import concourse.bass as bass
import concourse.tile as tile
from concourse import bacc, mybir
from concourse.bass_utils import run_bass_kernel_spmd

F32 = mybir.dt.float32
BF16 = mybir.dt.bfloat16
ACT = mybir.ActivationFunctionType

N_CORES = 8
B, T, C = 64, 64, 4096
H, HD = 64, 64
NTOK = (B // N_CORES) * T            # 512 tokens per core
O3 = 3 * C                           # 12288
NM = O3 // 128                       # 96 feature tiles of 128
NK = C // 128                        # 32 contraction tiles of 128
TCH = 128                            # attention token chunk
NGRP = 8                             # tokens per attention group


def _build_program(reps=1, stages=(1, 2, 3), att_mode="full"):
    nc = bacc.Bacc(
        "TRN2", target_bir_lowering=False, debug=False, num_devices=N_CORES
    )

    xT = nc.declare_dram_parameter("xT", [C, NTOK], BF16, isOutput=False)
    WaT = nc.declare_dram_parameter("WaT", [C, O3], BF16, isOutput=False)
    WpT = nc.declare_dram_parameter("WpT", [C, C], BF16, isOutput=False)
    b_att = nc.declare_dram_parameter("b_att", [128, NM], F32, isOutput=False)
    b_prj = nc.declare_dram_parameter("b_prj", [128, NK], F32, isOutput=False)
    maskT8 = nc.declare_dram_parameter("maskT8", [H, NGRP * H], BF16, isOutput=False)
    onesc = nc.declare_dram_parameter("onesc", [H, 1], BF16, isOutput=False)
    outT = nc.declare_dram_parameter("outT", [C, NTOK], F32, isOutput=True)

    with tile.TileContext(nc) as tc, ExitStack() as ctx:
        for rep in range(reps):
            with ExitStack() as rctx:
                _emit(
                    rctx, tc, xT, WaT, WpT, b_att, b_prj, maskT8, onesc,
                    outT, rep, stages, att_mode,
                )
    nc.compile()
    return nc


def _emit(ctx, tc, xT, WaT, WpT, b_att, b_prj, maskT8, onesc, outT, rep=0,
          stages=(1, 2, 3), att_mode="full"):
    nc = tc.nc
    R = f"r{rep}_"

    const = ctx.enter_context(tc.tile_pool(name=R + "const", bufs=1))
    b_att_sb = const.tile([128, NM], F32)
    nc.sync.dma_start(b_att_sb[:], b_att.ap())
    b_prj_sb = const.tile([128, NK], F32)
    nc.sync.dma_start(b_prj_sb[:], b_prj.ap())
    mask_sb = const.tile([H, NGRP * H], BF16)
    nc.sync.dma_start(mask_sb[:], maskT8.ap())
    ones_sb = const.tile([H, 1], BF16)
    nc.sync.dma_start(ones_sb[:], onesc.ap())

    dram = ctx.enter_context(tc.tile_pool(name=R + "dram", bufs=1, space="DRAM"))
    Qrot = dram.tile([HD, H * NTOK], BF16)    # [d, (h, t)]
    Krot = dram.tile([HD, H * NTOK], BF16)    # [d, (h, t)]
    Vrot = dram.tile([H, HD * NTOK], BF16)    # [h, (d, t)]
    Ydram = dram.tile([H, HD * NTOK], BF16)   # [i, (d, t)] == yT row-major

    q3 = Qrot.rearrange("d (h t) -> d h t", t=NTOK)
    k3 = Krot.rearrange("d (h t) -> d h t", t=NTOK)
    v3 = Vrot.rearrange("h (d t) -> h d t", t=NTOK)
    y3 = Ydram.rearrange("i (d t) -> i d t", t=NTOK)

    # ---------------- Stage 1: QKV + rotation to DRAM ----------------
    wpool = ctx.enter_context(tc.tile_pool(name=R + "wpool", bufs=3))
    ps1 = ctx.enter_context(tc.tile_pool(name=R + "ps1", bufs=2, space="PSUM"))
    secp = ctx.enter_context(tc.tile_pool(name=R + "secp", bufs=4))

    if 1 in stages:
        _emit_stage1(ctx, tc, R, xT, WaT, b_att_sb, wpool, ps1, secp,
                     q3, k3, v3)
    if 2 in stages:
        _emit_stage2(ctx, tc, R, mask_sb, ones_sb, q3, k3, v3, y3, att_mode)
    if 3 in stages:
        _emit_stage3(ctx, tc, R, WpT, b_prj_sb, wpool, ps1, Ydram, outT)


def _emit_stage1(ctx, tc, R, xT, WaT, b_att_sb, wpool, ps1, secp, q3, k3, v3):
    nc = tc.nc
    NSEG, TSEG = 2, NTOK // 2
    with tc.tile_pool(name=R + "xpool", bufs=1) as xpool:
        x_sb = xpool.tile([128, NK * NTOK], BF16)
        nc.sync.dma_start(
            x_sb.rearrange("p (k t) -> p k t", t=NTOK),
            xT.ap().rearrange("(k p) t -> p k t", p=128),
        )

        for seg in range(NSEG):
            tlo = seg * TSEG
            for m in range(NM):
                wa = wpool.tile(
                    [128, NK * 128], BF16, name=f"{R}wa{seg}_{m}", tag="wa"
                )
                nc.sync.dma_start(
                    wa.rearrange("p (k o) -> p k o", o=128),
                    WaT.ap()[:, m * 128 : (m + 1) * 128].rearrange(
                        "(k p) o -> p k o", p=128
                    ),
                )
                ps = ps1.tile(
                    [128, TSEG], F32, name=f"{R}ps{seg}_{m}", tag="ps"
                )
                for kc in range(NK):
                    nc.tensor.matmul(
                        ps[:],
                        wa[:, kc * 128 : (kc + 1) * 128],
                        x_sb[:, kc * NTOK + tlo : kc * NTOK + tlo + TSEG],
                        start=(kc == 0),
                        stop=(kc == NK - 1),
                    )
                sec = secp.tile(
                    [128, TSEG], BF16, name=f"{R}sec{seg}_{m}", tag="sec"
                )
                is_k = NK <= m < 2 * NK
                nc.scalar.activation(
                    sec[:],
                    ps[:],
                    ACT.Identity,
                    bias=b_att_sb[:, m : m + 1],
                    scale=0.125 if is_k else 1.0,
                )
                # rotate the two heads of this tile out to DRAM
                if m < NK:
                    dst = q3
                    h0 = m * 2
                elif m < 2 * NK:
                    dst = k3
                    h0 = (m - NK) * 2
                else:
                    dst = None
                    h0 = (m - 2 * NK) * 2
                for h2 in range(2):
                    h = h0 + h2
                    s2 = sec[h2 * 64 : h2 * 64 + 64, :]
                    if dst is not None:
                        nc.sync.dma_start(dst[:, h, tlo : tlo + TSEG], s2)
                    else:
                        nc.sync.dma_start(v3[h, :, tlo : tlo + TSEG], s2)


def _emit_stage2(ctx, tc, R, mask_sb, ones_sb, q3, k3, v3, y3,
                 att_mode="full"):
    nc = tc.nc
    qkp = ctx.enter_context(tc.tile_pool(name=R + "qkp", bufs=2))
    psA = ctx.enter_context(tc.tile_pool(name=R + "psA", bufs=3, space="PSUM"))
    psB = ctx.enter_context(tc.tile_pool(name=R + "psB", bufs=2, space="PSUM"))
    psC = ctx.enter_context(tc.tile_pool(name=R + "psC", bufs=1, space="PSUM"))
    atp = ctx.enter_context(tc.tile_pool(name=R + "atp", bufs=3))

    for c0 in range(0, NTOK, TCH):
        qr = qkp.tile([HD, H * TCH], BF16, name=f"{R}qr{c0}", tag="qr")
        kr = qkp.tile([HD, H * TCH], BF16, name=f"{R}kr{c0}", tag="kr")
        vr = qkp.tile([H, HD * TCH], BF16, name=f"{R}vr{c0}", tag="vr")
        nc.sync.dma_start(
            qr.rearrange("d (h t) -> d h t", t=TCH), q3[:, :, c0 : c0 + TCH]
        )
        nc.sync.dma_start(
            kr.rearrange("d (h t) -> d h t", t=TCH), k3[:, :, c0 : c0 + TCH]
        )
        nc.sync.dma_start(
            vr.rearrange("h (d t) -> h d t", t=TCH), v3[:, :, c0 : c0 + TCH]
        )
        kr3 = kr.rearrange("d (h t) -> d h t", t=TCH)
        qr3 = qr.rearrange("d (h t) -> d h t", t=TCH)
        vr3 = vr.rearrange("h (d t) -> h d t", t=TCH)
        ystc = qkp.tile([H, HD * TCH], BF16, name=f"{R}ystc{c0}", tag="ystc")
        ystc3 = ystc.rearrange("i (d t) -> i d t", t=TCH)
        # shared PSUM bank for softmax denominators (two groups by parity)
        psn_sh = psC.tile([H, 2 * NGRP], F32, name=f"{R}psn{c0}", tag="psn")

        def scores(g):
            """Scores + exp + causal mask for one 8-token group."""
            t0 = g * NGRP
            ps_s = psA.tile(
                [H, NGRP * H], F32, name=f"{R}pss{c0}_{g}", tag="pss"
            )
            for tl in range(NGRP):
                t = t0 + tl
                nc.tensor.matmul(
                    ps_s[:, tl * H : (tl + 1) * H],
                    kr3[:, :, t],
                    qr3[:, :, t],
                    start=True,
                    stop=True,
                )
            exp_sb = atp.tile(
                [H, NGRP * H], BF16, name=f"{R}exp{c0}_{g}", tag="exp"
            )
            nc.scalar.activation(exp_sb[:], ps_s[:], ACT.Exp)
            att = atp.tile(
                [H, NGRP * H], BF16, name=f"{R}att{c0}_{g}", tag="att"
            )
            nc.vector.tensor_mul(att[:], exp_sb[:], mask_sb[:])
            return g, att

        def finish(g, att):
            """y / denominator matmuls + normalized eviction into ystc."""
            t0 = g * NGRP
            ps_y = psB.tile(
                [H, NGRP * H], F32, name=f"{R}psy{c0}_{g}", tag="psy"
            )
            ps_n = psn_sh[:, (g % 2) * NGRP : (g % 2) * NGRP + NGRP]
            for tl in range(NGRP):
                a_t = att[:, tl * H : (tl + 1) * H]
                nc.tensor.matmul(
                    ps_n[:, tl : tl + 1], a_t, ones_sb[:],
                    start=True, stop=True,
                )
                nc.tensor.matmul(
                    ps_y[:, tl * H : (tl + 1) * H],
                    a_t,
                    vr3[:, :, t0 + tl],
                    start=True,
                    stop=True,
                )
            rs = atp.tile([H, NGRP], F32, name=f"{R}rs{c0}_{g}", tag="rs")
            nc.vector.reciprocal(rs[:], ps_n)
            # one batched eviction: ystc[:, :, t0:t0+8] = ps_y * rs[i, t]
            # (rs broadcast over d via a zero-step AP dim)
            rs_b = bass.AP(
                rs.tensor, rs.offset, [list(rs.ap[0]), [0, HD], [1, NGRP]]
            )
            nc.vector.tensor_mul(
                ystc3[:, :, t0 : t0 + NGRP],
                ps_y.rearrange("i (t d) -> i d t", d=HD),
                rs_b,
            )

        prev = None
        for g in range(TCH // NGRP):
            cur = scores(g)
            if prev is not None:
                finish(*prev)
            prev = cur
        finish(*prev)
        nc.sync.dma_start(y3[:, :, c0 : c0 + TCH], ystc3)


def _emit_stage3(ctx, tc, R, WpT, b_prj_sb, wpool, ps1, Ydram, outT):
    nc = tc.nc
    outp = ctx.enter_context(tc.tile_pool(name=R + "outp", bufs=3))
    with tc.tile_pool(name=R + "ypool", bufs=1) as ypool:
        yt_sb = ypool.tile([128, NK * NTOK], BF16)
        yv = yt_sb.rearrange("(p2 d) (k t) -> p2 d k t", p2=2, t=NTOK)
        dv = Ydram.rearrange("(k p2) (d t) -> p2 d k t", p2=2, t=NTOK)
        for c0 in range(0, NTOK, TCH):
            for p2 in range(2):
                nc.sync.dma_start(
                    yv[p2][:, :, c0 : c0 + TCH], dv[p2][:, :, c0 : c0 + TCH]
                )
        for mo in range(NK):
            wp = wpool.tile([128, NK * 128], BF16, name=f"{R}wp{mo}", tag="wa")
            nc.sync.dma_start(
                wp.rearrange("p (k o) -> p k o", o=128),
                WpT.ap()[:, mo * 128 : (mo + 1) * 128].rearrange(
                    "(k p) o -> p k o", p=128
                ),
            )
            ps = ps1.tile([128, NTOK], F32, name=f"{R}pso{mo}", tag="ps")
            for kc in range(NK):
                nc.tensor.matmul(
                    ps[:],
                    wp[:, kc * 128 : (kc + 1) * 128],
                    yt_sb[:, kc * NTOK : (kc + 1) * NTOK],
                    start=(kc == 0),
                    stop=(kc == NK - 1),
                )
            ob = outp.tile([128, NTOK], F32, name=f"{R}ob{mo}", tag="ob")
            nc.scalar.activation(
                ob[:], ps[:], ACT.Identity, bias=b_prj_sb[:, mo : mo + 1]
            )
            nc.sync.dma_start(outT.ap()[mo * 128 : (mo + 1) * 128, :], ob[:])


_PROGRAMS = {}


def _get_program(reps=1):
    if reps not in _PROGRAMS:
        _PROGRAMS[reps] = _build_program(reps)
    return _PROGRAMS[reps]


def _host_inputs(x, W_atten, b_atten, W_proj, b_proj):
    bf = ml_dtypes.bfloat16
    x = np.asarray(x, np.float32).reshape(B, T, C)
    WaT = np.ascontiguousarray(np.asarray(W_atten, np.float32).T).astype(bf)
    WpT = np.ascontiguousarray(np.asarray(W_proj, np.float32).T).astype(bf)
    ba = np.asarray(b_atten, np.float32).copy()
    ba[C : 2 * C] *= 0.125  # fold the 1/sqrt(hd) score scale into k bias
    b_att_h = np.ascontiguousarray(ba.reshape(NM, 128).T)
    b_prj_h = np.ascontiguousarray(
        np.asarray(b_proj, np.float32).reshape(NK, 128).T
    )
    mask = np.tril(np.ones((H, H), np.float32))  # keep j<=i
    maskT8_h = np.ascontiguousarray(np.tile(mask.T, (1, NGRP))).astype(bf)
    ones_h = np.ones((H, 1), bf)

    shard_b = B // N_CORES
    in_maps = []
    for i in range(N_CORES):
        xs = x[i * shard_b : (i + 1) * shard_b].reshape(NTOK, C)
        xT_i = np.ascontiguousarray(xs.T).astype(bf)
        in_maps.append(
            {
                "xT": xT_i,
                "WaT": WaT,
                "WpT": WpT,
                "b_att": b_att_h,
                "b_prj": b_prj_h,
                "maskT8": maskT8_h,
                "onesc": ones_h,
            }
        )
    return in_maps


def run(inputs, trace=False):
    nc = _get_program()
    in_maps = _host_inputs(**inputs)
    res = run_bass_kernel_spmd(
        nc, in_maps, list(range(N_CORES)), trace=trace
    )
    shard_b = B // N_CORES
    out = np.empty((B, T, C), dtype=np.float32)
    for i in range(N_CORES):
        oT = np.asarray(res.results[i]["outT"], np.float32)  # (C, NTOK)
        out[i * shard_b : (i + 1) * shard_b] = oT.T.reshape(shard_b, T, C)
    return out, res


def kernel(x, W_atten, b_atten, W_proj, b_proj):
    out, _ = run(
        dict(
            x=x,
            W_atten=W_atten,
            b_atten=b_atten,
            W_proj=W_proj,
            b_proj=b_proj,
        )
    )
    return out



# revision 2
# speedup vs baseline: 5.9304x; 5.9304x over previous
"""nn_Attention_30511447671564 — Trainium2 Bass kernel.

Head-mixing attention block. Shapes (hardcoded): B=64, T=64, C=4096,
H=64, hd=64, rank=1.  For every token (b,t): attention mixes the 64
heads (HxH scores, causal over head index).

    qkv = x @ W_atten^T + b_atten                  (B,T,3C)
    per-token: s[i,j] = q_i . k_j / 8  (i,j heads, causal j<=i)
               att = softmax_j(s);  y_i = sum_j att[i,j] v_j
    out = y @ W_proj^T + b_proj                    (B,T,C)

Distribution: pure data-parallel — 8 cores x 512 tokens, no collectives.

Per-core device pipeline (all matmuls bf16 with fp32 PSUM accumulation):
  1. QKV "Form 2": qkvT[o, t] = sum_c WaT[c, o] * xT[c, t]; bias (and the
     1/8 score scale, folded into k) applied on PSUM eviction.
  2. Rotation: evicted feature tiles [(h,d) x t] are DMA'd to DRAM bounce
     tensors Qrot/Krot [d, h, t] and Vrot [h, d, t] (contiguous runs).
  3. Attention in 128-token chunks: per token, 64x64 matmuls
     sT = K_t^T-style scores with heads contracted over d; exp (no max
     subtraction — scores are O(10), safe in fp32); causal mask applied
     multiplicatively after exp; softmax denominators via a ones-column
     matmul off the same stationary attT; normalization folded into the
     PSUM eviction scale. y is written (d-major) and DMA'd to Ydram.
  4. Projection "Form 2" from Ydram, bias on eviction, transposed output
     outT[c, t] written to DRAM; host un-transposes.
"""

import numpy as np
import ml_dtypes
from contextlib import ExitStack

import concourse.bass as bass
import concourse.tile as tile
from concourse import bacc, mybir
from concourse.bass_utils import run_bass_kernel_spmd

F32 = mybir.dt.float32
BF16 = mybir.dt.bfloat16
ACT = mybir.ActivationFunctionType

N_CORES = 8
B, T, C = 64, 64, 4096
H, HD = 64, 64
NTOK = (B // N_CORES) * T            # 512 tokens per core
O3 = 3 * C                           # 12288
NM = O3 // 128                       # 96 feature tiles of 128
NK = C // 128                        # 32 contraction tiles of 128
TCH = 128                            # attention token chunk
NGRP = 8                             # tokens per attention group


def _build_program(reps=1, stages=(1, 2, 3), att_mode="full"):
    nc = bacc.Bacc(
        "TRN2", target_bir_lowering=False, debug=False, num_devices=N_CORES
    )

    xT = nc.declare_dram_parameter("xT", [C, NTOK], BF16, isOutput=False)
    WaT = nc.declare_dram_parameter("WaT", [C, O3], BF16, isOutput=False)
    WpT = nc.declare_dram_parameter("WpT", [C, C], BF16, isOutput=False)
    b_att = nc.declare_dram_parameter("b_att", [128, NM], F32, isOutput=False)
    b_prj = nc.declare_dram_parameter("b_prj", [128, NK], F32, isOutput=False)
    maskT8 = nc.declare_dram_parameter("maskT8", [H, NGRP * H], BF16, isOutput=False)
    onesc = nc.declare_dram_parameter("onesc", [H, 1], BF16, isOutput=False)
    outT = nc.declare_dram_parameter("outT", [C, NTOK], F32, isOutput=True)

    with tile.TileContext(nc) as tc, ExitStack() as ctx:
        for rep in range(reps):
            with ExitStack() as rctx:
                _emit(
                    rctx, tc, xT, WaT, WpT, b_att, b_prj, maskT8, onesc,
                    outT, rep, stages, att_mode,
                )
    nc.compile()
    return nc


def _emit(ctx, tc, xT, WaT, WpT, b_att, b_prj, maskT8, onesc, outT, rep=0,
          stages=(1, 2, 3), att_mode="full"):
    nc = tc.nc
    R = f"r{rep}_"

    const = ctx.enter_context(tc.tile_pool(name=R + "const", bufs=1))
    b_att_sb = const.tile([128, NM], F32)
    nc.sync.dma_start(b_att_sb[:], b_att.ap())
    b_prj_sb = const.tile([128, NK], F32)
    nc.sync.dma_start(b_prj_sb[:], b_prj.ap())
    mask_sb = const.tile([H, NGRP * H], BF16)
    nc.sync.dma_start(mask_sb[:], maskT8.ap())
    ones_sb = const.tile([H, 1], BF16)
    nc.sync.dma_start(ones_sb[:], onesc.ap())

    dram = ctx.enter_context(tc.tile_pool(name=R + "dram", bufs=1, space="DRAM"))
    Qrot = dram.tile([HD, H * NTOK], BF16)    # [d, (h, t)]
    Krot = dram.tile([HD, H * NTOK], BF16)    # [d, (h, t)]
    Vrot = dram.tile([H, HD * NTOK], BF16)    # [h, (d, t)]
    Ydram = dram.tile([H, HD * NTOK], BF16)   # [i, (d, t)] == yT row-major

    q3 = Qrot.rearrange("d (h t) -> d h t", t=NTOK)
    k3 = Krot.rearrange("d (h t) -> d h t", t=NTOK)
    v3 = Vrot.rearrange("h (d t) -> h d t", t=NTOK)
    y3 = Ydram.rearrange("i (d t) -> i d t", t=NTOK)

    # ---------------- Stage 1: QKV + rotation to DRAM ----------------
    wpool = ctx.enter_context(tc.tile_pool(name=R + "wpool", bufs=3))
    ps1 = ctx.enter_context(tc.tile_pool(name=R + "ps1", bufs=2, space="PSUM"))
    secp = ctx.enter_context(tc.tile_pool(name=R + "secp", bufs=4))

    if 1 in stages:
        _emit_stage1(ctx, tc, R, xT, WaT, b_att_sb, wpool, ps1, secp,
                     q3, k3, v3)
    if 2 in stages:
        _emit_stage2(ctx, tc, R, mask_sb, ones_sb, q3, k3, v3, y3, att_mode)
    if 3 in stages:
        _emit_stage3(ctx, tc, R, WpT, b_prj_sb, wpool, ps1, Ydram, outT)


def _emit_stage1(ctx, tc, R, xT, WaT, b_att_sb, wpool, ps1, secp, q3, k3, v3):
    nc = tc.nc
    NSEG, TSEG = 2, NTOK // 2
    with tc.tile_pool(name=R + "xpool", bufs=1) as xpool:
        x_sb = xpool.tile([128, NK * NTOK], BF16)
        nc.sync.dma_start(
            x_sb.rearrange("p (k t) -> p k t", t=NTOK),
            xT.ap().rearrange("(k p) t -> p k t", p=128),
        )

        for seg in range(NSEG):
            tlo = seg * TSEG
            for m in range(NM):
                wa = wpool.tile(
                    [128, NK * 128], BF16, name=f"{R}wa{seg}_{m}", tag="wa"
                )
                nc.sync.dma_start(
                    wa.rearrange("p (k o) -> p k o", o=128),
                    WaT.ap()[:, m * 128 : (m + 1) * 128].rearrange(
                        "(k p) o -> p k o", p=128
                    ),
                )
                ps = ps1.tile(
                    [128, TSEG], F32, name=f"{R}ps{seg}_{m}", tag="ps"
                )
                for kc in range(NK):
                    nc.tensor.matmul(
                        ps[:],
                        wa[:, kc * 128 : (kc + 1) * 128],
                        x_sb[:, kc * NTOK + tlo : kc * NTOK + tlo + TSEG],
                        start=(kc == 0),
                        stop=(kc == NK - 1),
                    )
                sec = secp.tile(
                    [128, TSEG], BF16, name=f"{R}sec{seg}_{m}", tag="sec"
                )
                is_k = NK <= m < 2 * NK
                nc.scalar.activation(
                    sec[:],
                    ps[:],
                    ACT.Identity,
                    bias=b_att_sb[:, m : m + 1],
                    scale=0.125 if is_k else 1.0,
                )
                # rotate the two heads of this tile out to DRAM
                if m < NK:
                    dst = q3
                    h0 = m * 2
                elif m < 2 * NK:
                    dst = k3
                    h0 = (m - NK) * 2
                else:
                    dst = None
                    h0 = (m - 2 * NK) * 2
                for h2 in range(2):
                    h = h0 + h2
                    s2 = sec[h2 * 64 : h2 * 64 + 64, :]
                    if dst is not None:
                        nc.sync.dma_start(dst[:, h, tlo : tlo + TSEG], s2)
                    else:
                        nc.sync.dma_start(v3[h, :, tlo : tlo + TSEG], s2)


def _emit_stage2(ctx, tc, R, mask_sb, ones_sb, q3, k3, v3, y3,
                 att_mode="full"):
    nc = tc.nc
    qkp = ctx.enter_context(tc.tile_pool(name=R + "qkp", bufs=2))
    psA = ctx.enter_context(tc.tile_pool(name=R + "psA", bufs=3, space="PSUM"))
    psB = ctx.enter_context(tc.tile_pool(name=R + "psB", bufs=2, space="PSUM"))
    psC = ctx.enter_context(tc.tile_pool(name=R + "psC", bufs=1, space="PSUM"))
    atp = ctx.enter_context(tc.tile_pool(name=R + "atp", bufs=3))

    for c0 in range(0, NTOK, TCH):
        qr = qkp.tile([HD, H * TCH], BF16, name=f"{R}qr{c0}", tag="qr")
        kr = qkp.tile([HD, H * TCH], BF16, name=f"{R}kr{c0}", tag="kr")
        vr = qkp.tile([H, HD * TCH], BF16, name=f"{R}vr{c0}", tag="vr")
        nc.sync.dma_start(
            qr.rearrange("d (h t) -> d h t", t=TCH), q3[:, :, c0 : c0 + TCH]
        )
        nc.sync.dma_start(
            kr.rearrange("d (h t) -> d h t", t=TCH), k3[:, :, c0 : c0 + TCH]
        )
        nc.sync.dma_start(
            vr.rearrange("h (d t) -> h d t", t=TCH), v3[:, :, c0 : c0 + TCH]
        )
        kr3 = kr.rearrange("d (h t) -> d h t", t=TCH)
        qr3 = qr.rearrange("d (h t) -> d h t", t=TCH)
        vr3 = vr.rearrange("h (d t) -> h d t", t=TCH)
        ystc = qkp.tile([H, HD * TCH], BF16, name=f"{R}ystc{c0}", tag="ystc")
        ystc3 = ystc.rearrange("i (d t) -> i d t", t=TCH)
        # shared PSUM bank for softmax denominators (two groups by parity)
        psn_sh = psC.tile([H, 2 * NGRP], F32, name=f"{R}psn{c0}", tag="psn")

        def scores(g):
            """Scores + exp + causal mask for one 8-token group."""
            t0 = g * NGRP
            ps_s = psA.tile(
                [H, NGRP * H], F32, name=f"{R}pss{c0}_{g}", tag="pss"
            )
            for tl in range(NGRP):
                t = t0 + tl
                nc.tensor.matmul(
                    ps_s[:, tl * H : (tl + 1) * H],
                    kr3[:, :, t],
                    qr3[:, :, t],
                    start=True,
                    stop=True,
                )
            exp_sb = atp.tile(
                [H, NGRP * H], BF16, name=f"{R}exp{c0}_{g}", tag="exp"
            )
            nc.scalar.activation(exp_sb[:], ps_s[:], ACT.Exp)
            att = atp.tile(
                [H, NGRP * H], BF16, name=f"{R}att{c0}_{g}", tag="att"
            )
            nc.vector.tensor_mul(att[:], exp_sb[:], mask_sb[:])
            return g, att

        def finish(g, att):
            """y / denominator matmuls + normalized eviction into ystc."""
            t0 = g * NGRP
            ps_y = psB.tile(
                [H, NGRP * H], F32, name=f"{R}psy{c0}_{g}", tag="psy"
            )
            ps_n = psn_sh[:, (g % 2) * NGRP : (g % 2) * NGRP + NGRP]
            for tl in range(NGRP):
                a_t = att[:, tl * H : (tl + 1) * H]
                nc.tensor.matmul(
                    ps_n[:, tl : tl + 1], a_t, ones_sb[:],
                    start=True, stop=True,
                )
                nc.tensor.matmul(
                    ps_y[:, tl * H : (tl + 1) * H],
                    a_t,
                    vr3[:, :, t0 + tl],
                    start=True,
                    stop=True,
                )
            rs = atp.tile([H, NGRP], F32, name=f"{R}rs{c0}_{g}", tag="rs")
            nc.vector.reciprocal(rs[:], ps_n)
            # one batched eviction: ystc[:, :, t0:t0+8] = ps_y * rs[i, t]
            # (rs broadcast over d via a zero-step AP dim)
            rs_b = bass.AP(
                rs.tensor, rs.offset, [list(rs.ap[0]), [0, HD], [1, NGRP]]
            )
            nc.vector.tensor_mul(
                ystc3[:, :, t0 : t0 + NGRP],
                ps_y.rearrange("i (t d) -> i d t", d=HD),
                rs_b,
            )

        prev = None
        for g in range(TCH // NGRP):
            cur = scores(g)
            if prev is not None:
                finish(*prev)
            prev = cur
        finish(*prev)
        nc.sync.dma_start(y3[:, :, c0 : c0 + TCH], ystc3)


def _emit_stage3(ctx, tc, R, WpT, b_prj_sb, wpool, ps1, Ydram, outT):
    nc = tc.nc
    outp = ctx.enter_context(tc.tile_pool(name=R + "outp", bufs=3))
    with tc.tile_pool(name=R + "ypool", bufs=1) as ypool:
        yt_sb = ypool.tile([128, NK * NTOK], BF16)
        yv = yt_sb.rearrange("(p2 d) (k t) -> p2 d k t", p2=2, t=NTOK)
        dv = Ydram.rearrange("(k p2) (d t) -> p2 d k t", p2=2, t=NTOK)
        for c0 in range(0, NTOK, TCH):
            for p2 in range(2):
                nc.sync.dma_start(
                    yv[p2][:, :, c0 : c0 + TCH], dv[p2][:, :, c0 : c0 + TCH]
                )
        for mo in range(NK):
            wp = wpool.tile([128, NK * 128], BF16, name=f"{R}wp{mo}", tag="wa")
            nc.sync.dma_start(
                wp.rearrange("p (k o) -> p k o", o=128),
                WpT.ap()[:, mo * 128 : (mo + 1) * 128].rearrange(
                    "(k p) o -> p k o", p=128
                ),
            )
            ps = ps1.tile([128, NTOK], F32, name=f"{R}pso{mo}", tag="ps")
            for kc in range(NK):
                nc.tensor.matmul(
                    ps[:],
                    wp[:, kc * 128 : (kc + 1) * 128],
                    yt_sb[:, kc * NTOK : (kc + 1) * NTOK],
                    start=(kc == 0),
                    stop=(kc == NK - 1),
                )
            ob = outp.tile([128, NTOK], F32, name=f"{R}ob{mo}", tag="ob")
            nc.scalar.activation(
                ob[:], ps[:], ACT.Identity, bias=b_prj_sb[:, mo : mo + 1]
            )
            nc.sync.dma_start(outT.ap()[mo * 128 : (mo + 1) * 128, :], ob[:])


_PROGRAMS = {}


def _get_program(reps=1):
    if reps not in _PROGRAMS:
        _PROGRAMS[reps] = _build_program(reps)
    return _PROGRAMS[reps]


def _host_inputs(x, W_atten, b_atten, W_proj, b_proj):
    bf = ml_dtypes.bfloat16
    x = np.asarray(x, np.float32).reshape(B, T, C)
    WaT = np.ascontiguousarray(np.asarray(W_atten, np.float32).T).astype(bf)
    WpT = np.ascontiguousarray(np.asarray(W_proj, np.float32).T).astype(bf)
    ba = np.asarray(b_atten, np.float32).copy()
    ba[C : 2 * C] *= 0.125  # fold the 1/sqrt(hd) score scale into k bias
    b_att_h = np.ascontiguousarray(ba.reshape(NM, 128).T)
    b_prj_h = np.ascontiguousarray(
        np.asarray(b_proj, np.float32).reshape(NK, 128).T
    )
    mask = np.tril(np.ones((H, H), np.float32))  # keep j<=i
    maskT8_h = np.ascontiguousarray(np.tile(mask.T, (1, NGRP))).astype(bf)
    ones_h = np.ones((H, 1), bf)

    shard_b = B // N_CORES
    in_maps = []
    for i in range(N_CORES):
        xs = x[i * shard_b : (i + 1) * shard_b].reshape(NTOK, C)
        xT_i = np.ascontiguousarray(xs.T).astype(bf)
        in_maps.append(
            {
                "xT": xT_i,
                "WaT": WaT,
                "WpT": WpT,
                "b_att": b_att_h,
                "b_prj": b_prj_h,
                "maskT8": maskT8_h,
                "onesc": ones_h,
            }
        )
    return in_maps


def _host_outputs(results):
    shard_b = B // N_CORES
    out = np.empty((B, T, C), dtype=np.float32)
    for i in range(N_CORES):
        oT = np.asarray(results[i]["outT"], np.float32)  # (C, NTOK)
        out[i * shard_b : (i + 1) * shard_b] = oT.T.reshape(shard_b, T, C)
    return out


def run(inputs, trace=False):
    nc = _get_program()
    in_maps = _host_inputs(**inputs)
    res = run_bass_kernel_spmd(
        nc, in_maps, list(range(N_CORES)), trace=trace
    )
    return _host_outputs(res.results), res


def kernel(x, W_atten, b_atten, W_proj, b_proj):
    out, _ = run(
        dict(
            x=x,
            W_atten=W_atten,
            b_atten=b_atten,
            W_proj=W_proj,
            b_proj=b_proj,
        )
    )
    return out



# revision 3
# speedup vs baseline: 11.7630x; 1.9835x over previous
"""nn_Attention_30511447671564 — Trainium2 Bass kernel (v2).

Head-mixing attention block. Shapes (hardcoded): B=64, T=64, C=4096,
H=64, hd=64, rank=1.  For every token (b,t): attention mixes the 64
heads (HxH scores, causal over head index).

    qkv = x @ W_atten^T + b_atten                  (B,T,3C)
    per-token: s[i,j] = q_i . k_j / 8  (i,j heads, causal j<=i)
               att = softmax_j(s);  y_i = sum_j att[i,j] v_j
    out = y @ W_proj^T + b_proj                    (B,T,C)

Distribution: pure data-parallel — 8 cores x 512 tokens, no collectives.

v2 changes vs v1 (trace-driven; v1 was DMA-bound at 146 GB/s with 270B
packets and a serialized DMA queue):
  * Weights host-prepacked so each 128x(32*128) stationary tile is one
    contiguous 1MB DRAM read (8KB per partition) — one DMA per tile.
  * Stage 1 in a single 512-token pass (halves weight traffic vs two
    256-token segments).
  * The 1/8 score scale is folded into the K rows of W_atten/b_atten on
    the host; PSUM evictions are bias-adds on the (idle) Vector engine.
  * DMAs split across two hardware queues: loads on the Sync queue,
    bounce-buffer writes on the Scalar queue (queues serialize per
    engine; v1 put everything on Sync).
  * Stage 1 emits K features first, then Q, then V, so the attention
    stage's gather DMAs overlap the tail of stage 1.
  * Attention: tokens t and t+256 are processed as a pair in opposite
    PE-array quadrants (tile_position (0,0)/(64,64)) — q/k/v for the
    two half-chunks are stacked on partitions 0:64 / 64:128.  The
    softmax denominator comes free as a 65th "ones" column of v
    (killing the per-token denominator matmul + LDWEIGHTS of v1).
"""

import numpy as np
import ml_dtypes
from contextlib import ExitStack

import concourse.bass as bass
import concourse.tile as tile
from concourse import bacc, mybir
from concourse.bass_utils import run_bass_kernel_spmd

F32 = mybir.dt.float32
BF16 = mybir.dt.bfloat16
ACT = mybir.ActivationFunctionType

N_CORES = 8
B, T, C = 64, 64, 4096
H, HD = 64, 64
NTOK = (B // N_CORES) * T            # 512 tokens per core
O3 = 3 * C                           # 12288
NM = O3 // 128                       # 96 qkv feature tiles of 128
NK = C // 128                        # 32 contraction tiles of 128
THALF = NTOK // 2                    # 256: attention half-chunk length
GP = 4                               # token-pairs per attention group


def _bcast(ap, dims):
    """AP broadcast helper: keep partition dim, append given free dims."""
    return bass.AP(ap.tensor, ap.offset, [list(ap.ap[0])] + dims)


def _build_program(reps=1, stages=(1, 2, 3)):
    nc = bacc.Bacc(
        "TRN2", target_bir_lowering=False, debug=False, num_devices=N_CORES
    )

    xl = nc.declare_dram_parameter("xl", [128, NK * NTOK], BF16, isOutput=False)
    Wal = nc.declare_dram_parameter("Wal", [NM * 128, NK * 128], BF16, isOutput=False)
    Wpl = nc.declare_dram_parameter("Wpl", [NK * 128, NK * 128], BF16, isOutput=False)
    b_att = nc.declare_dram_parameter("b_att", [128, NM], F32, isOutput=False)
    b_prj = nc.declare_dram_parameter("b_prj", [128, NK], F32, isOutput=False)
    maskT2 = nc.declare_dram_parameter("maskT2", [128, H], BF16, isOutput=False)
    outT = nc.declare_dram_parameter("outT", [C, NTOK], F32, isOutput=True)

    with tile.TileContext(nc) as tc, ExitStack() as ctx:
        for rep in range(reps):
            with ExitStack() as rctx:
                _emit(rctx, tc, xl, Wal, Wpl, b_att, b_prj, maskT2, outT,
                      rep, stages)
    nc.compile()
    return nc


def _emit(ctx, tc, xl, Wal, Wpl, b_att, b_prj, maskT2, outT, rep=0,
          stages=(1, 2, 3)):
    nc = tc.nc
    R = f"r{rep}_"

    const = ctx.enter_context(tc.tile_pool(name=R + "const", bufs=1))
    b_att_sb = const.tile([128, NM], F32)
    nc.sync.dma_start(b_att_sb[:], b_att.ap())
    b_prj_sb = const.tile([128, NK], F32)
    nc.sync.dma_start(b_prj_sb[:], b_prj.ap())
    mask_sb = const.tile([128, H], BF16)
    nc.sync.dma_start(mask_sb[:], maskT2.ap())

    dram = ctx.enter_context(tc.tile_pool(name=R + "dram", bufs=1, space="DRAM"))
    Qrot = dram.tile([HD, H * NTOK], BF16)    # [d, (h, t)]
    Krot = dram.tile([HD, H * NTOK], BF16)    # [d, (h, t)]
    Vrot = dram.tile([H, HD * NTOK], BF16)    # [h, (d, t)]
    Ydram = dram.tile([H, HD * NTOK], BF16)   # [i, (d, t)]

    q3 = Qrot.rearrange("d (h t) -> d h t", t=NTOK)
    k3 = Krot.rearrange("d (h t) -> d h t", t=NTOK)
    v3 = Vrot.rearrange("h (d t) -> h d t", t=NTOK)
    y3 = Ydram.rearrange("i (d t) -> i d t", t=NTOK)

    wpool = ctx.enter_context(tc.tile_pool(name=R + "wpool", bufs=3))
    ps1 = ctx.enter_context(tc.tile_pool(name=R + "ps1", bufs=2, space="PSUM"))

    if 1 in stages:
        _emit_stage1(ctx, tc, R, xl, Wal, b_att_sb, wpool, ps1, q3, k3, v3)
    if 2 in stages:
        _emit_stage2(ctx, tc, R, mask_sb, q3, k3, v3, y3)
    if 3 in stages:
        _emit_stage3(ctx, tc, R, Wpl, b_prj_sb, wpool, ps1, Ydram, outT)


def _emit_stage1(ctx, tc, R, xl, Wal, b_att_sb, wpool, ps1, q3, k3, v3):
    nc = tc.nc
    with tc.tile_pool(name=R + "xpool", bufs=1) as xpool, \
         tc.tile_pool(name=R + "secp", bufs=4) as secp:
        x_sb = xpool.tile([128, NK * NTOK], BF16)
        nc.sync.dma_start(x_sb[:], xl.ap())

        # K features first, then Q, then V: lets stage 2's q/k gathers
        # start while stage 1 is still computing V.
        morder = list(range(NK, 2 * NK)) + list(range(NK)) + \
            list(range(2 * NK, NM))
        for m in morder:
            wa = wpool.tile([128, NK * 128], BF16, name=f"{R}wa{m}", tag="wa")
            nc.sync.dma_start(wa[:], Wal.ap()[m * 128:(m + 1) * 128, :])
            ps = ps1.tile([128, NTOK], F32, name=f"{R}ps{m}", tag="ps")
            for kc in range(NK):
                nc.tensor.matmul(
                    ps[:],
                    wa[:, kc * 128:(kc + 1) * 128],
                    x_sb[:, kc * NTOK:(kc + 1) * NTOK],
                    start=(kc == 0),
                    stop=(kc == NK - 1),
                )
            sec = secp.tile([128, NTOK], BF16, name=f"{R}sec{m}", tag="sec")
            bb = b_att_sb[:, m:m + 1]
            nc.vector.tensor_add(sec[:], ps[:], _bcast(bb, [[0, NTOK]]))
            # rotate the two heads of this tile out to the DRAM bounce
            if m < NK:
                dst3, h0 = q3, 2 * m
            elif m < 2 * NK:
                dst3, h0 = k3, 2 * (m - NK)
            else:
                dst3, h0 = None, 2 * (m - 2 * NK)
            for h2 in range(2):
                s2 = sec[h2 * 64:h2 * 64 + 64, :]
                if dst3 is not None:
                    nc.scalar.dma_start(dst3[:, h0 + h2, :], s2)
                else:
                    nc.scalar.dma_start(v3[h0 + h2, :, :], s2)


def _emit_stage2(ctx, tc, R, mask_sb, q3, k3, v3, y3):
    nc = tc.nc
    qkp = ctx.enter_context(tc.tile_pool(name=R + "qkp", bufs=1))
    psA = ctx.enter_context(tc.tile_pool(name=R + "psA", bufs=3, space="PSUM"))
    psB = ctx.enter_context(tc.tile_pool(name=R + "psB", bufs=3, space="PSUM"))
    atp = ctx.enter_context(tc.tile_pool(name=R + "atp", bufs=4))

    # stacked layouts: partitions 0:64 = tokens [0,256), 64:128 = [256,512)
    qp = qkp.tile([128, H * THALF], BF16)     # [(p2,d), (i, t')]
    kp = qkp.tile([128, H * THALF], BF16)
    v4 = qkp.tile([128, 65 * THALF], BF16)    # [(p2,j), (d|ones, t')]
    ystc = qkp.tile([128, HD * THALF], BF16)  # [(p2,i), (d, t')]
    qpv = qp.rearrange("p (i t) -> p i t", t=THALF)
    kpv = kp.rearrange("p (i t) -> p i t", t=THALF)
    v4v = v4.rearrange("p (d t) -> p d t", t=THALF)
    ystc3 = ystc.rearrange("p (d t) -> p d t", t=THALF)

    nc.sync.dma_start(qpv[0:64], q3[:, :, 0:THALF])
    nc.sync.dma_start(qpv[64:128], q3[:, :, THALF:NTOK])
    nc.sync.dma_start(kpv[0:64], k3[:, :, 0:THALF])
    nc.sync.dma_start(kpv[64:128], k3[:, :, THALF:NTOK])
    nc.vector.memset(v4v[:, 64, :], 1.0)      # fused-denominator ones row
    nc.sync.dma_start(v4v[0:64, 0:64, :], v3[:, :, 0:THALF])
    nc.sync.dma_start(v4v[64:128, 0:64, :], v3[:, :, THALF:NTOK])

    def scores(g):
        ps_s = psA.tile([128, GP * H], F32, name=f"{R}pss{g}", tag="pss")
        for pl in range(GP):
            t = g * GP + pl
            nc.tensor.matmul(
                ps_s[0:64, pl * H:(pl + 1) * H],
                kpv[0:64, :, t], qpv[0:64, :, t],
                start=True, stop=True, tile_position=(0, 0),
            )
            nc.tensor.matmul(
                ps_s[64:128, pl * H:(pl + 1) * H],
                kpv[64:128, :, t], qpv[64:128, :, t],
                start=True, stop=True, tile_position=(64, 64),
            )
        exp_sb = atp.tile([128, GP * H], BF16, name=f"{R}exp{g}", tag="exp")
        nc.scalar.activation(exp_sb[:], ps_s[:], ACT.Exp)
        att = atp.tile([128, GP * H], BF16, name=f"{R}att{g}", tag="att")
        nc.vector.tensor_mul(
            att[:], exp_sb[:], _bcast(mask_sb[:, 0:1], [[0, GP], [1, H]])
        )
        return g, att

    def finish(g, att):
        ps_y = psB.tile([128, GP * 65], F32, name=f"{R}psy{g}", tag="psy")
        py3 = ps_y.rearrange("p (pr e) -> p pr e", e=65)
        for pl in range(GP):
            t = g * GP + pl
            a0 = att[0:64, pl * H:(pl + 1) * H]
            a1 = att[64:128, pl * H:(pl + 1) * H]
            nc.tensor.matmul(
                py3[0:64, pl, :], a0, v4v[0:64, :, t],
                start=True, stop=True, tile_position=(0, 0),
            )
            nc.tensor.matmul(
                py3[64:128, pl, :], a1, v4v[64:128, :, t],
                start=True, stop=True, tile_position=(64, 64),
            )
        rs = atp.tile([128, GP], F32, name=f"{R}rs{g}", tag="rs")
        nc.vector.reciprocal(rs[:], py3[:, :, 64])
        # ystc[:, d, g*GP + pr] = ps_y[:, pr, d] * rs[:, pr]
        src = bass.AP(ps_y.tensor, ps_y.offset,
                      [list(ps_y.ap[0]), [1, HD], [65, GP]])
        nc.vector.tensor_mul(
            ystc3[:, :, g * GP:(g + 1) * GP],
            src,
            _bcast(rs[:, 0:1], [[0, HD], [1, GP]]),
        )

    prev = None
    for g in range(THALF // GP):
        cur = scores(g)
        if prev is not None:
            finish(*prev)
        prev = cur
    finish(*prev)

    nc.scalar.dma_start(y3[:, :, 0:THALF], ystc3[0:64])
    nc.scalar.dma_start(y3[:, :, THALF:NTOK], ystc3[64:128])


def _emit_stage3(ctx, tc, R, Wpl, b_prj_sb, wpool, ps1, Ydram, outT):
    nc = tc.nc
    outp = ctx.enter_context(tc.tile_pool(name=R + "outp", bufs=3))
    with tc.tile_pool(name=R + "ypool", bufs=1) as ypool:
        yt_sb = ypool.tile([128, NK * NTOK], BF16)
        yv = yt_sb.rearrange("(p2 d) (k t) -> p2 d k t", p2=2, t=NTOK)
        dv = Ydram.rearrange("(k p2) (d t) -> p2 d k t", p2=2, t=NTOK)
        for p2 in range(2):
            nc.sync.dma_start(yv[p2], dv[p2])
        for mo in range(NK):
            wp = wpool.tile([128, NK * 128], BF16, name=f"{R}wp{mo}", tag="wa")
            nc.sync.dma_start(wp[:], Wpl.ap()[mo * 128:(mo + 1) * 128, :])
            ps = ps1.tile([128, NTOK], F32, name=f"{R}pso{mo}", tag="ps")
            for kc in range(NK):
                nc.tensor.matmul(
                    ps[:],
                    wp[:, kc * 128:(kc + 1) * 128],
                    yt_sb[:, kc * NTOK:(kc + 1) * NTOK],
                    start=(kc == 0),
                    stop=(kc == NK - 1),
                )
            ob = outp.tile([128, NTOK], F32, name=f"{R}ob{mo}", tag="ob")
            bb = b_prj_sb[:, mo:mo + 1]
            nc.vector.tensor_add(ob[:], ps[:], _bcast(bb, [[0, NTOK]]))
            nc.scalar.dma_start(outT.ap()[mo * 128:(mo + 1) * 128, :], ob[:])


_PROGRAMS = {}


def _get_program(reps=1):
    if reps not in _PROGRAMS:
        _PROGRAMS[reps] = _build_program(reps)
    return _PROGRAMS[reps]


def _host_inputs(x, W_atten, b_atten, W_proj, b_proj):
    bf = ml_dtypes.bfloat16
    x = np.asarray(x, np.float32).reshape(B, T, C)
    Wa = np.asarray(W_atten, np.float32).copy()
    ba = np.asarray(b_atten, np.float32).copy()
    Wa[C:2 * C] *= 0.125  # fold the 1/sqrt(hd) score scale into K
    ba[C:2 * C] *= 0.125
    # Wal[m*128+p, k*128+o] = Wa[m*128+o, k*128+p] (contiguous 1MB tiles)
    Wal = np.ascontiguousarray(
        Wa.reshape(NM, 128, NK, 128).transpose(0, 3, 2, 1)
    ).astype(bf).reshape(NM * 128, NK * 128)
    Wpl = np.ascontiguousarray(
        np.asarray(W_proj, np.float32).reshape(NK, 128, NK, 128)
        .transpose(0, 3, 2, 1)
    ).astype(bf).reshape(NK * 128, NK * 128)
    b_att_h = np.ascontiguousarray(ba.reshape(NM, 128).T)
    b_prj_h = np.ascontiguousarray(
        np.asarray(b_proj, np.float32).reshape(NK, 128).T
    )
    mask = np.tril(np.ones((H, H), np.float32))  # keep j<=i
    maskT2_h = np.ascontiguousarray(np.vstack([mask.T, mask.T])).astype(bf)

    shard_b = B // N_CORES
    in_maps = []
    for i in range(N_CORES):
        xs = x[i * shard_b:(i + 1) * shard_b].reshape(NTOK, C)
        xl_i = np.ascontiguousarray(
            xs.reshape(NTOK, NK, 128).transpose(2, 1, 0)
        ).astype(bf).reshape(128, NK * NTOK)
        in_maps.append(
            {
                "xl": xl_i,
                "Wal": Wal,
                "Wpl": Wpl,
                "b_att": b_att_h,
                "b_prj": b_prj_h,
                "maskT2": maskT2_h,
            }
        )
    return in_maps


def _host_outputs(results):
    shard_b = B // N_CORES
    out = np.empty((B, T, C), dtype=np.float32)
    for i in range(N_CORES):
        oT = np.asarray(results[i]["outT"], np.float32)  # (C, NTOK)
        out[i * shard_b:(i + 1) * shard_b] = oT.T.reshape(shard_b, T, C)
    return out


def run(inputs, trace=False):
    nc = _get_program()
    in_maps = _host_inputs(**inputs)
    res = run_bass_kernel_spmd(
        nc, in_maps, list(range(N_CORES)), trace=trace
    )
    return _host_outputs(res.results), res


def kernel(x, W_atten, b_atten, W_proj, b_proj):
    out, _ = run(
        dict(
            x=x,
            W_atten=W_atten,
            b_atten=b_atten,
            W_proj=W_proj,
            b_proj=b_proj,
        )
    )
    return out


# revision 8
# speedup vs baseline: 11.9094x; 1.0124x over previous
"""nn_Attention_30511447671564 — Trainium2 Bass kernel (v2).

Head-mixing attention block. Shapes (hardcoded): B=64, T=64, C=4096,
H=64, hd=64, rank=1.  For every token (b,t): attention mixes the 64
heads (HxH scores, causal over head index).

    qkv = x @ W_atten^T + b_atten                  (B,T,3C)
    per-token: s[i,j] = q_i . k_j / 8  (i,j heads, causal j<=i)
               att = softmax_j(s);  y_i = sum_j att[i,j] v_j
    out = y @ W_proj^T + b_proj                    (B,T,C)

Distribution: pure data-parallel — 8 cores x 512 tokens, no collectives.

v2 changes vs v1 (trace-driven; v1 was DMA-bound at 146 GB/s with 270B
packets and a serialized DMA queue):
  * Weights host-prepacked so each 128x(32*128) stationary tile is one
    contiguous 1MB DRAM read (8KB per partition) — one DMA per tile.
  * Stage 1 in a single 512-token pass (halves weight traffic vs two
    256-token segments).
  * The 1/8 score scale is folded into the K rows of W_atten/b_atten on
    the host; PSUM evictions are bias-adds on the (idle) Vector engine.
  * DMAs split across two hardware queues: loads on the Sync queue,
    bounce-buffer writes on the Scalar queue (queues serialize per
    engine; v1 put everything on Sync).
  * Stage 1 emits K features first, then Q, then V, so the attention
    stage's gather DMAs overlap the tail of stage 1.
  * Attention: tokens t and t+256 are processed as a pair in opposite
    PE-array quadrants (tile_position (0,0)/(64,64)) — q/k/v for the
    two half-chunks are stacked on partitions 0:64 / 64:128.  The
    softmax denominator comes free as a 65th "ones" column of v
    (killing the per-token denominator matmul + LDWEIGHTS of v1).
"""

import numpy as np
import ml_dtypes
from contextlib import ExitStack

import concourse.bass as bass
import concourse.tile as tile
from concourse import bacc, mybir
from concourse.bass_utils import run_bass_kernel_spmd

F32 = mybir.dt.float32
BF16 = mybir.dt.bfloat16
ACT = mybir.ActivationFunctionType

N_CORES = 8
B, T, C = 64, 64, 4096
H, HD = 64, 64
NTOK = (B // N_CORES) * T            # 512 tokens per core
O3 = 3 * C                           # 12288
NM = O3 // 128                       # 96 qkv feature tiles of 128
NK = C // 128                        # 32 contraction tiles of 128
THALF = NTOK // 2                    # 256: attention half-chunk length
GP = 4                               # token-pairs per attention group


def _bcast(ap, dims):
    """AP broadcast helper: keep partition dim, append given free dims."""
    return bass.AP(ap.tensor, ap.offset, [list(ap.ap[0])] + dims)


def _build_program(reps=1, stages=(1, 2, 3)):
    nc = bacc.Bacc(
        "TRN2", target_bir_lowering=False, debug=False, num_devices=N_CORES
    )

    xl = nc.declare_dram_parameter("xl", [128, NK * NTOK], BF16, isOutput=False)
    Wal = nc.declare_dram_parameter("Wal", [NM * 128, NK * 128], BF16, isOutput=False)
    Wpl = nc.declare_dram_parameter("Wpl", [NK * 128, NK * 128], BF16, isOutput=False)
    b_att = nc.declare_dram_parameter("b_att", [128, NM], F32, isOutput=False)
    b_prj = nc.declare_dram_parameter("b_prj", [128, NK], F32, isOutput=False)
    maskT2 = nc.declare_dram_parameter("maskT2", [128, H], BF16, isOutput=False)
    outT = nc.declare_dram_parameter("outT", [C, NTOK], F32, isOutput=True)

    with tile.TileContext(nc) as tc, ExitStack() as ctx:
        for rep in range(reps):
            with ExitStack() as rctx:
                _emit(rctx, tc, xl, Wal, Wpl, b_att, b_prj, maskT2, outT,
                      rep, stages)
    nc.compile()
    return nc


def _emit(ctx, tc, xl, Wal, Wpl, b_att, b_prj, maskT2, outT, rep=0,
          stages=(1, 2, 3)):
    nc = tc.nc
    R = f"r{rep}_"

    const = ctx.enter_context(tc.tile_pool(name=R + "const", bufs=1))
    b_att_sb = const.tile([128, NM], F32)
    nc.sync.dma_start(b_att_sb[:], b_att.ap())
    b_prj_sb = const.tile([128, NK], F32)
    nc.sync.dma_start(b_prj_sb[:], b_prj.ap())
    mask_sb = const.tile([128, H], BF16)
    nc.sync.dma_start(mask_sb[:], maskT2.ap())

    dram = ctx.enter_context(tc.tile_pool(name=R + "dram", bufs=1, space="DRAM"))
    Qrot = dram.tile([HD, H * NTOK], BF16)    # [d, (h, t)]
    Krot = dram.tile([HD, H * NTOK], BF16)    # [d, (h, t)]
    Vrot = dram.tile([H, HD * NTOK], BF16)    # [h, (d, t)]
    Ydram = dram.tile([H, HD * NTOK], BF16)   # [i, (d, t)]

    q3 = Qrot.rearrange("d (h t) -> d h t", t=NTOK)
    k3 = Krot.rearrange("d (h t) -> d h t", t=NTOK)
    v3 = Vrot.rearrange("h (d t) -> h d t", t=NTOK)
    y3 = Ydram.rearrange("i (d t) -> i d t", t=NTOK)

    wpool = ctx.enter_context(tc.tile_pool(name=R + "wpool", bufs=4))
    ps1 = ctx.enter_context(tc.tile_pool(name=R + "ps1", bufs=2, space="PSUM"))

    # attention-stage tiles exist up front so stage 1 can gather into them
    qkp = ctx.enter_context(tc.tile_pool(name=R + "qkp", bufs=1))
    qp = qkp.tile([128, H * THALF], BF16)     # [(p2,d), (i, t')]
    kp = qkp.tile([128, H * THALF], BF16)
    v4 = qkp.tile([128, 65 * THALF], BF16)    # [(p2,j), (d|ones, t')]
    ystc = qkp.tile([128, HD * THALF], BF16)  # [(p2,i), (d, t')]
    qpv = qp.rearrange("p (i t) -> p i t", t=THALF)
    kpv = kp.rearrange("p (i t) -> p i t", t=THALF)
    v4v = v4.rearrange("p (d t) -> p d t", t=THALF)
    ystc3 = ystc.rearrange("p (d t) -> p d t", t=THALF)
    nc.vector.memset(v4v[:, 64, :], 1.0)      # fused-denominator ones row

    def gathers(c):
        """Gather 8 heads' worth of q/k/v produced by the last 4 stage-1
        tiles into the stacked attention layouts (Sync queue)."""
        if c < 8:
            h0, src, dstv = 8 * c, k3, kpv
        elif c < 16:
            h0 = 8 * (c - 8)
            nc.sync.dma_start(
                v4v[h0:h0 + 8, 0:64, :], v3[h0:h0 + 8, :, 0:THALF]
            )
            nc.sync.dma_start(
                v4v[64 + h0:64 + h0 + 8, 0:64, :],
                v3[h0:h0 + 8, :, THALF:NTOK],
            )
            return
        else:
            h0, src, dstv = 8 * (c - 16), q3, qpv
        nc.sync.dma_start(dstv[0:64, h0:h0 + 8, :], src[:, h0:h0 + 8, 0:THALF])
        nc.sync.dma_start(
            dstv[64:128, h0:h0 + 8, :], src[:, h0:h0 + 8, THALF:NTOK]
        )

    if 1 in stages:
        _emit_stage1(ctx, tc, R, xl, Wal, b_att_sb, wpool, ps1, q3, k3, v3,
                     gathers)

    # prefetch the first two projection weight tiles during attention
    wp_pre = {}
    for mo in range(2):
        wp = wpool.tile([128, NK * 128], BF16, name=f"{R}wp{mo}", tag="wa")
        nc.sync.dma_start(wp[:], Wpl.ap()[mo * 128:(mo + 1) * 128, :])
        wp_pre[mo] = wp

    ypool = ctx.enter_context(tc.tile_pool(name=R + "ypool", bufs=1))
    yt_sb = ypool.tile([128, NK * NTOK], BF16)
    yv = yt_sb.rearrange("(p2 d) (k t) -> p2 d k t", p2=2, t=NTOK)
    dv = Ydram.rearrange("(k p2) (d t) -> p2 d k t", p2=2, t=NTOK)

    def y_flush(q):
        """Write attention-output quarter q to the DRAM bounce (Scalar
        queue) and immediately load it back in stage-3 layout (Sync)."""
        a, b = q * 64, (q + 1) * 64
        nc.scalar.dma_start(y3[:, :, a:b], ystc3[0:64, :, a:b])
        nc.scalar.dma_start(
            y3[:, :, THALF + a:THALF + b], ystc3[64:128, :, a:b]
        )
        for p2 in range(2):
            nc.sync.dma_start(yv[p2][:, :, a:b], dv[p2][:, :, a:b])
            nc.sync.dma_start(
                yv[p2][:, :, THALF + a:THALF + b],
                dv[p2][:, :, THALF + a:THALF + b],
            )

    if 2 in stages:
        _emit_stage2(ctx, tc, R, mask_sb, qpv, kpv, v4v, ystc3, y_flush)
    if 3 in stages:
        _emit_stage3(ctx, tc, R, Wpl, b_prj_sb, wpool, ps1, yt_sb, outT,
                     wp_pre)


def _emit_stage1(ctx, tc, R, xl, Wal, b_att_sb, wpool, ps1, q3, k3, v3,
                 gathers):
    nc = tc.nc
    with tc.tile_pool(name=R + "xpool", bufs=1) as xpool, \
         tc.tile_pool(name=R + "secp", bufs=4) as secp:
        x_sb = xpool.tile([128, NK * NTOK], BF16)
        # split the x load so m-tile 0's first matmuls start early
        XSP = 8
        for xc in range(0, NK, XSP):
            nc.sync.dma_start(
                x_sb[:, xc * NTOK:(xc + XSP) * NTOK],
                xl.ap()[:, xc * NTOK:(xc + XSP) * NTOK],
            )

        # K, V, Q tile order; after every 4 tiles the 8 heads they
        # produced are gathered into the attention-stage layouts (the
        # gather DMAs interleave with weight loads on the Sync queue).
        morder = list(range(NK, 2 * NK)) + list(range(2 * NK, NM)) + \
            list(range(NK))
        for mi, m in enumerate(morder):
            wa = wpool.tile([128, NK * 128], BF16, name=f"{R}wa{m}", tag="wa")
            nc.sync.dma_start(wa[:], Wal.ap()[m * 128:(m + 1) * 128, :])
            ps = ps1.tile([128, NTOK], F32, name=f"{R}ps{m}", tag="ps")
            for kc in range(NK):
                nc.tensor.matmul(
                    ps[:],
                    wa[:, kc * 128:(kc + 1) * 128],
                    x_sb[:, kc * NTOK:(kc + 1) * NTOK],
                    start=(kc == 0),
                    stop=(kc == NK - 1),
                )
            sec = secp.tile([128, NTOK], BF16, name=f"{R}sec{m}", tag="sec")
            bb = b_att_sb[:, m:m + 1]
            nc.vector.tensor_add(sec[:], ps[:], _bcast(bb, [[0, NTOK]]))
            # rotate the two heads of this tile out to the DRAM bounce
            if m < NK:
                dst3, h0 = q3, 2 * m
            elif m < 2 * NK:
                dst3, h0 = k3, 2 * (m - NK)
            else:
                dst3, h0 = None, 2 * (m - 2 * NK)
            for h2 in range(2):
                s2 = sec[h2 * 64:h2 * 64 + 64, :]
                if dst3 is not None:
                    nc.scalar.dma_start(dst3[:, h0 + h2, :], s2)
                else:
                    nc.scalar.dma_start(v3[h0 + h2, :, :], s2)
            if mi % 4 == 3:
                gathers(mi // 4)


def _emit_stage2(ctx, tc, R, mask_sb, qpv, kpv, v4v, ystc3, y_flush):
    nc = tc.nc
    psA = ctx.enter_context(tc.tile_pool(name=R + "psA", bufs=3, space="PSUM"))
    psB = ctx.enter_context(tc.tile_pool(name=R + "psB", bufs=3, space="PSUM"))
    atp = ctx.enter_context(tc.tile_pool(name=R + "atp", bufs=4))

    def scores(g):
        ps_s = psA.tile([128, GP * H], F32, name=f"{R}pss{g}", tag="pss")
        for pl in range(GP):
            t = g * GP + pl
            nc.tensor.matmul(
                ps_s[0:64, pl * H:(pl + 1) * H],
                kpv[0:64, :, t], qpv[0:64, :, t],
                start=True, stop=True, tile_position=(0, 0),
            )
            nc.tensor.matmul(
                ps_s[64:128, pl * H:(pl + 1) * H],
                kpv[64:128, :, t], qpv[64:128, :, t],
                start=True, stop=True, tile_position=(64, 64),
            )
        exp_sb = atp.tile([128, GP * H], BF16, name=f"{R}exp{g}", tag="exp")
        nc.scalar.activation(exp_sb[:], ps_s[:], ACT.Exp)
        att = atp.tile([128, GP * H], BF16, name=f"{R}att{g}", tag="att")
        nc.vector.tensor_mul(
            att[:], exp_sb[:], _bcast(mask_sb[:, 0:1], [[0, GP], [1, H]])
        )
        return g, att

    def finish(g, att):
        ps_y = psB.tile([128, GP * 65], F32, name=f"{R}psy{g}", tag="psy")
        py3 = ps_y.rearrange("p (pr e) -> p pr e", e=65)
        for pl in range(GP):
            t = g * GP + pl
            a0 = att[0:64, pl * H:(pl + 1) * H]
            a1 = att[64:128, pl * H:(pl + 1) * H]
            nc.tensor.matmul(
                py3[0:64, pl, :], a0, v4v[0:64, :, t],
                start=True, stop=True, tile_position=(0, 0),
            )
            nc.tensor.matmul(
                py3[64:128, pl, :], a1, v4v[64:128, :, t],
                start=True, stop=True, tile_position=(64, 64),
            )
        rs = atp.tile([128, GP], F32, name=f"{R}rs{g}", tag="rs")
        nc.vector.reciprocal(rs[:], py3[:, :, 64])
        # ystc[:, d, g*GP + pr] = ps_y[:, pr, d] * rs[:, pr]
        src = bass.AP(ps_y.tensor, ps_y.offset,
                      [list(ps_y.ap[0]), [1, HD], [65, GP]])
        nc.vector.tensor_mul(
            ystc3[:, :, g * GP:(g + 1) * GP],
            src,
            _bcast(rs[:, 0:1], [[0, HD], [1, GP]]),
        )

    prev = None
    ngrp = THALF // GP
    qq = ngrp // 4
    for g in range(ngrp):
        cur = scores(g)
        if prev is not None:
            finish(*prev)
        prev = cur
        # finish(g-1) has run; quarter q is complete once finish(q*qq+qq-1)
        # has been emitted, i.e. at g == (q+1)*qq
        if g % qq == 0 and g > 0:
            y_flush(g // qq - 1)
    finish(*prev)
    y_flush(3)


def _emit_stage3(ctx, tc, R, Wpl, b_prj_sb, wpool, ps1, yt_sb, outT, wp_pre):
    nc = tc.nc
    outp = ctx.enter_context(tc.tile_pool(name=R + "outp", bufs=3))
    for mo in range(NK):
        if mo in wp_pre:
            wp = wp_pre[mo]
        else:
            wp = wpool.tile([128, NK * 128], BF16, name=f"{R}wp{mo}", tag="wa")
            nc.sync.dma_start(wp[:], Wpl.ap()[mo * 128:(mo + 1) * 128, :])
        ps = ps1.tile([128, NTOK], F32, name=f"{R}pso{mo}", tag="ps")
        for kc in range(NK):
            nc.tensor.matmul(
                ps[:],
                wp[:, kc * 128:(kc + 1) * 128],
                yt_sb[:, kc * NTOK:(kc + 1) * NTOK],
                start=(kc == 0),
                stop=(kc == NK - 1),
            )
        ob = outp.tile([128, NTOK], F32, name=f"{R}ob{mo}", tag="ob")
        bb = b_prj_sb[:, mo:mo + 1]
        nc.vector.tensor_add(ob[:], ps[:], _bcast(bb, [[0, NTOK]]))
        nc.scalar.dma_start(outT.ap()[mo * 128:(mo + 1) * 128, :], ob[:])


_PROGRAMS = {}


def _get_program(reps=1):
    if reps not in _PROGRAMS:
        _PROGRAMS[reps] = _build_program(reps)
    return _PROGRAMS[reps]


def _host_inputs(x, W_atten, b_atten, W_proj, b_proj):
    bf = ml_dtypes.bfloat16
    x = np.asarray(x, np.float32).reshape(B, T, C)
    Wa = np.asarray(W_atten, np.float32).copy()
    ba = np.asarray(b_atten, np.float32).copy()
    Wa[C:2 * C] *= 0.125  # fold the 1/sqrt(hd) score scale into K
    ba[C:2 * C] *= 0.125
    # Wal[m*128+p, k*128+o] = Wa[m*128+o, k*128+p] (contiguous 1MB tiles)
    Wal = np.ascontiguousarray(
        Wa.reshape(NM, 128, NK, 128).transpose(0, 3, 2, 1)
    ).astype(bf).reshape(NM * 128, NK * 128)
    Wpl = np.ascontiguousarray(
        np.asarray(W_proj, np.float32).reshape(NK, 128, NK, 128)
        .transpose(0, 3, 2, 1)
    ).astype(bf).reshape(NK * 128, NK * 128)
    b_att_h = np.ascontiguousarray(ba.reshape(NM, 128).T)
    b_prj_h = np.ascontiguousarray(
        np.asarray(b_proj, np.float32).reshape(NK, 128).T
    )
    mask = np.tril(np.ones((H, H), np.float32))  # keep j<=i
    maskT2_h = np.ascontiguousarray(np.vstack([mask.T, mask.T])).astype(bf)

    shard_b = B // N_CORES
    in_maps = []
    for i in range(N_CORES):
        xs = x[i * shard_b:(i + 1) * shard_b].reshape(NTOK, C)
        xl_i = np.ascontiguousarray(
            xs.reshape(NTOK, NK, 128).transpose(2, 1, 0)
        ).astype(bf).reshape(128, NK * NTOK)
        in_maps.append(
            {
                "xl": xl_i,
                "Wal": Wal,
                "Wpl": Wpl,
                "b_att": b_att_h,
                "b_prj": b_prj_h,
                "maskT2": maskT2_h,
            }
        )
    return in_maps


def _host_outputs(results):
    shard_b = B // N_CORES
    out = np.empty((B, T, C), dtype=np.float32)
    for i in range(N_CORES):
        oT = np.asarray(results[i]["outT"], np.float32)  # (C, NTOK)
        out[i * shard_b:(i + 1) * shard_b] = oT.T.reshape(shard_b, T, C)
    return out


def run(inputs, trace=False):
    nc = _get_program()
    in_maps = _host_inputs(**inputs)
    res = run_bass_kernel_spmd(
        nc, in_maps, list(range(N_CORES)), trace=trace
    )
    return _host_outputs(res.results), res


def kernel(x, W_atten, b_atten, W_proj, b_proj):
    out, _ = run(
        dict(
            x=x,
            W_atten=W_atten,
            b_atten=b_atten,
            W_proj=W_proj,
            b_proj=b_proj,
        )
    )
    return out


# revision 16
# speedup vs baseline: 12.2111x; 1.0253x over previous
"""nn_Attention_30511447671564 — Trainium2 Bass kernel (v2).

Head-mixing attention block. Shapes (hardcoded): B=64, T=64, C=4096,
H=64, hd=64, rank=1.  For every token (b,t): attention mixes the 64
heads (HxH scores, causal over head index).

    qkv = x @ W_atten^T + b_atten                  (B,T,3C)
    per-token: s[i,j] = q_i . k_j / 8  (i,j heads, causal j<=i)
               att = softmax_j(s);  y_i = sum_j att[i,j] v_j
    out = y @ W_proj^T + b_proj                    (B,T,C)

Distribution: pure data-parallel — 8 cores x 512 tokens, no collectives.

v2 changes vs v1 (trace-driven; v1 was DMA-bound at 146 GB/s with 270B
packets and a serialized DMA queue):
  * Weights host-prepacked so each 128x(32*128) stationary tile is one
    contiguous 1MB DRAM read (8KB per partition) — one DMA per tile.
  * Stage 1 in a single 512-token pass (halves weight traffic vs two
    256-token segments).
  * The 1/8 score scale is folded into the K rows of W_atten/b_atten on
    the host; PSUM evictions are bias-adds on the (idle) Vector engine.
  * DMAs split across two hardware queues: loads on the Sync queue,
    bounce-buffer writes on the Scalar queue (queues serialize per
    engine; v1 put everything on Sync).
  * Stage 1 emits K features first, then Q, then V, so the attention
    stage's gather DMAs overlap the tail of stage 1.
  * Attention: tokens t and t+256 are processed as a pair in opposite
    PE-array quadrants (tile_position (0,0)/(64,64)) — q/k/v for the
    two half-chunks are stacked on partitions 0:64 / 64:128.  The
    softmax denominator comes free as a 65th "ones" column of v
    (killing the per-token denominator matmul + LDWEIGHTS of v1).
"""

import numpy as np
import ml_dtypes
from contextlib import ExitStack

import concourse.bass as bass
import concourse.tile as tile
from concourse import bacc, mybir
from concourse.bass_utils import run_bass_kernel_spmd

F32 = mybir.dt.float32
BF16 = mybir.dt.bfloat16
ACT = mybir.ActivationFunctionType

N_CORES = 8
B, T, C = 64, 64, 4096
H, HD = 64, 64
NTOK = (B // N_CORES) * T            # 512 tokens per core
O3 = 3 * C                           # 12288
NM = O3 // 128                       # 96 qkv feature tiles of 128
NK = C // 128                        # 32 contraction tiles of 128
THALF = NTOK // 2                    # 256: attention half-chunk length
GP = 4                               # token-pairs per attention group


def _bcast(ap, dims):
    """AP broadcast helper: keep partition dim, append given free dims."""
    return bass.AP(ap.tensor, ap.offset, [list(ap.ap[0])] + dims)


def _build_program(reps=1, stages=(1, 2, 3)):
    nc = bacc.Bacc(
        "TRN2", target_bir_lowering=False, debug=False, num_devices=N_CORES
    )

    xl = nc.declare_dram_parameter("xl", [128, NK * NTOK], BF16, isOutput=False)
    Wal = nc.declare_dram_parameter("Wal", [NM * 128, NK * 128], BF16, isOutput=False)
    Wpl = nc.declare_dram_parameter("Wpl", [NK * 128, NK * 128], BF16, isOutput=False)
    b_att = nc.declare_dram_parameter("b_att", [128, NM], F32, isOutput=False)
    b_prj = nc.declare_dram_parameter("b_prj", [128, NK], F32, isOutput=False)
    maskT2 = nc.declare_dram_parameter("maskT2", [128, H], BF16, isOutput=False)
    outT = nc.declare_dram_parameter("outT", [C, NTOK], F32, isOutput=True)

    with tile.TileContext(nc) as tc, ExitStack() as ctx:
        for rep in range(reps):
            with ExitStack() as rctx:
                _emit(rctx, tc, xl, Wal, Wpl, b_att, b_prj, maskT2, outT,
                      rep, stages)
    nc.compile()
    return nc


def _emit(ctx, tc, xl, Wal, Wpl, b_att, b_prj, maskT2, outT, rep=0,
          stages=(1, 2, 3)):
    nc = tc.nc
    R = f"r{rep}_"

    const = ctx.enter_context(tc.tile_pool(name=R + "const", bufs=1))
    b_att_sb = const.tile([128, NM], F32)
    nc.sync.dma_start(b_att_sb[:], b_att.ap())
    b_prj_sb = const.tile([128, NK], F32)
    nc.sync.dma_start(b_prj_sb[:], b_prj.ap())
    mask_sb = const.tile([128, H], BF16)
    nc.sync.dma_start(mask_sb[:], maskT2.ap())
    # prewarm the scalar engine's Exp table set (~2.7us) before stage 2
    warm = const.tile([1, 1], F32)
    nc.scalar.activation(warm[:], mask_sb[0:1, 0:1], ACT.Exp)

    dram = ctx.enter_context(tc.tile_pool(name=R + "dram", bufs=1, space="DRAM"))
    Qrot = dram.tile([HD, H * NTOK], BF16)    # [d, (h, t)]
    Krot = dram.tile([HD, H * NTOK], BF16)    # [d, (h, t)]
    Vrot = dram.tile([H, HD * NTOK], BF16)    # [h, (d, t)]
    Ydram = dram.tile([H, HD * NTOK], BF16)   # [i, (d, t)]

    q3 = Qrot.rearrange("d (h t) -> d h t", t=NTOK)
    k3 = Krot.rearrange("d (h t) -> d h t", t=NTOK)
    v3 = Vrot.rearrange("h (d t) -> h d t", t=NTOK)
    y3 = Ydram.rearrange("i (d t) -> i d t", t=NTOK)

    wpool = ctx.enter_context(tc.tile_pool(name=R + "wpool", bufs=4))
    ps1 = ctx.enter_context(tc.tile_pool(name=R + "ps1", bufs=2, space="PSUM"))

    # attention-stage tiles exist up front so stage 1 can gather into them
    qkp = ctx.enter_context(tc.tile_pool(name=R + "qkp", bufs=1))
    qp = qkp.tile([128, H * THALF], BF16)     # [(p2,d), (i, t')]
    kp = qkp.tile([128, H * THALF], BF16)
    v4 = qkp.tile([128, 65 * THALF], BF16)    # [(p2,j), (d|ones, t')]
    ystc = qkp.tile([128, HD * THALF], BF16)  # [(p2,i), (d, t')]
    qpv = qp.rearrange("p (i t) -> p i t", t=THALF)
    kpv = kp.rearrange("p (i t) -> p i t", t=THALF)
    v4v = v4.rearrange("p (d t) -> p d t", t=THALF)
    ystc3 = ystc.rearrange("p (d t) -> p d t", t=THALF)
    nc.vector.memset(v4v[:, 64, :], 1.0)      # fused-denominator ones row

    def gathers(c):
        """Gather 8 heads' worth of q/k/v produced by the last 4 stage-1
        tiles into the stacked attention layouts.  q/k chunks write
        across 64 partitions (fast, Sync queue); v chunks concentrate
        32KB on 8 partitions (partition-write-bound, ~12us) so they go
        on the Scalar queue in 4-head pieces to not starve the weight
        loads."""
        if c < 8:
            h0, src, dstv = 8 * c, k3, kpv
        elif c < 16:
            # DMA duration scales with per-partition bytes (32KB per head
            # partition here), so chunking by heads only serializes it:
            # issue the whole v gather as 2 DMAs spanning all 64 head
            # partitions (~12us each, Scalar queue, overlaps the Q phase).
            if c == 15:
                nc.scalar.dma_start(v4v[0:64, 0:64, :], v3[:, :, 0:THALF])
                nc.scalar.dma_start(
                    v4v[64:128, 0:64, :], v3[:, :, THALF:NTOK]
                )
            return
        else:
            h0, src, dstv = 8 * (c - 16), q3, qpv
        nc.sync.dma_start(dstv[0:64, h0:h0 + 8, :], src[:, h0:h0 + 8, 0:THALF])
        nc.sync.dma_start(
            dstv[64:128, h0:h0 + 8, :], src[:, h0:h0 + 8, THALF:NTOK]
        )

    if 1 in stages:
        _emit_stage1(ctx, tc, R, xl, Wal, b_att_sb, wpool, ps1, q3, k3, v3,
                     gathers)

    # prefetch the first two projection weight tiles during attention
    wp_pre = {}
    for mo in range(2):
        wp = wpool.tile([128, NK * 128], BF16, name=f"{R}wp{mo}", tag="wa")
        nc.sync.dma_start(wp[:], Wpl.ap()[mo * 128:(mo + 1) * 128, :])
        wp_pre[mo] = wp

    ypool = ctx.enter_context(tc.tile_pool(name=R + "ypool", bufs=1))
    yt_sb = ypool.tile([128, NK * NTOK], BF16)
    yv = yt_sb.rearrange("(p2 d) (k t) -> p2 d k t", p2=2, t=NTOK)
    dv = Ydram.rearrange("(k p2) (d t) -> p2 d k t", p2=2, t=NTOK)

    def y_flush(q):
        """Write attention-output eighth q to the DRAM bounce (Scalar
        queue) and immediately load it back in stage-3 layout (Sync)."""
        a, b = q * 32, (q + 1) * 32
        nc.scalar.dma_start(y3[:, :, a:b], ystc3[0:64, :, a:b])
        nc.scalar.dma_start(
            y3[:, :, THALF + a:THALF + b], ystc3[64:128, :, a:b]
        )
        for p2 in range(2):
            nc.sync.dma_start(yv[p2][:, :, a:b], dv[p2][:, :, a:b])
            nc.sync.dma_start(
                yv[p2][:, :, THALF + a:THALF + b],
                dv[p2][:, :, THALF + a:THALF + b],
            )

    if 2 in stages:
        _emit_stage2(ctx, tc, R, mask_sb, qpv, kpv, v4v, ystc3, y_flush)
    if 3 in stages:
        _emit_stage3(ctx, tc, R, Wpl, b_prj_sb, wpool, ps1, yt_sb, outT,
                     wp_pre)


def _emit_stage1(ctx, tc, R, xl, Wal, b_att_sb, wpool, ps1, q3, k3, v3,
                 gathers):
    nc = tc.nc
    with tc.tile_pool(name=R + "xpool", bufs=1) as xpool, \
         tc.tile_pool(name=R + "secp", bufs=4) as secp:
        x_sb = xpool.tile([128, NK * NTOK], BF16)
        # first x chunk ahead of the first weight tile; rest behind it
        XSP = 8
        nc.sync.dma_start(
            x_sb[:, 0:XSP * NTOK], xl.ap()[:, 0:XSP * NTOK]
        )

        # K, V, Q tile order; after every 4 tiles the 8 heads they
        # produced are gathered into the attention-stage layouts (the
        # gather DMAs interleave with weight loads on the Sync queue).
        morder = list(range(NK, 2 * NK)) + list(range(2 * NK, NM)) + \
            list(range(NK))
        for mi, m in enumerate(morder):
            wa = wpool.tile([128, NK * 128], BF16, name=f"{R}wa{m}", tag="wa")
            nc.sync.dma_start(wa[:], Wal.ap()[m * 128:(m + 1) * 128, :])
            if mi == 0:
                for xc in range(XSP, NK, XSP):
                    nc.sync.dma_start(
                        x_sb[:, xc * NTOK:(xc + XSP) * NTOK],
                        xl.ap()[:, xc * NTOK:(xc + XSP) * NTOK],
                    )
            ps = ps1.tile([128, NTOK], F32, name=f"{R}ps{m}", tag="ps")
            for kc in range(NK):
                nc.tensor.matmul(
                    ps[:],
                    wa[:, kc * 128:(kc + 1) * 128],
                    x_sb[:, kc * NTOK:(kc + 1) * NTOK],
                    start=(kc == 0),
                    stop=(kc == NK - 1),
                )
            sec = secp.tile([128, NTOK], BF16, name=f"{R}sec{m}", tag="sec")
            bb = b_att_sb[:, m:m + 1]
            nc.vector.tensor_add(sec[:], ps[:], _bcast(bb, [[0, NTOK]]))
            # rotate the two heads of this tile out to the DRAM bounce
            if m < NK:
                dst3, h0 = q3, 2 * m
            elif m < 2 * NK:
                dst3, h0 = k3, 2 * (m - NK)
            else:
                dst3, h0 = None, 2 * (m - 2 * NK)
            for h2 in range(2):
                s2 = sec[h2 * 64:h2 * 64 + 64, :]
                if dst3 is not None:
                    nc.scalar.dma_start(dst3[:, h0 + h2, :], s2)
                else:
                    nc.scalar.dma_start(v3[h0 + h2, :, :], s2)
            if mi % 4 == 3:
                gathers(mi // 4)


def _emit_stage2(ctx, tc, R, mask_sb, qpv, kpv, v4v, ystc3, y_flush):
    nc = tc.nc
    psA = ctx.enter_context(tc.tile_pool(name=R + "psA", bufs=3, space="PSUM"))
    psB = ctx.enter_context(tc.tile_pool(name=R + "psB", bufs=3, space="PSUM"))
    atp = ctx.enter_context(tc.tile_pool(name=R + "atp", bufs=4))

    def scores(g):
        ps_s = psA.tile([128, GP * H], F32, name=f"{R}pss{g}", tag="pss")
        for pl in range(GP):
            t = g * GP + pl
            nc.tensor.matmul(
                ps_s[0:64, pl * H:(pl + 1) * H],
                kpv[0:64, :, t], qpv[0:64, :, t],
                start=True, stop=True, tile_position=(0, 0),
            )
            nc.tensor.matmul(
                ps_s[64:128, pl * H:(pl + 1) * H],
                kpv[64:128, :, t], qpv[64:128, :, t],
                start=True, stop=True, tile_position=(64, 64),
            )
        exp_sb = atp.tile([128, GP * H], BF16, name=f"{R}exp{g}", tag="exp")
        nc.scalar.activation(exp_sb[:], ps_s[:], ACT.Exp)
        att = atp.tile([128, GP * H], BF16, name=f"{R}att{g}", tag="att")
        nc.vector.tensor_mul(
            att[:], exp_sb[:], _bcast(mask_sb[:, 0:1], [[0, GP], [1, H]])
        )
        return g, att

    def finish(g, att):
        ps_y = psB.tile([128, GP * 65], F32, name=f"{R}psy{g}", tag="psy")
        py3 = ps_y.rearrange("p (pr e) -> p pr e", e=65)
        for pl in range(GP):
            t = g * GP + pl
            a0 = att[0:64, pl * H:(pl + 1) * H]
            a1 = att[64:128, pl * H:(pl + 1) * H]
            nc.tensor.matmul(
                py3[0:64, pl, :], a0, v4v[0:64, :, t],
                start=True, stop=True, tile_position=(0, 0),
            )
            nc.tensor.matmul(
                py3[64:128, pl, :], a1, v4v[64:128, :, t],
                start=True, stop=True, tile_position=(64, 64),
            )
        rs = atp.tile([128, GP], F32, name=f"{R}rs{g}", tag="rs")
        nc.vector.reciprocal(rs[:], py3[:, :, 64])
        # ystc[:, d, g*GP + pr] = ps_y[:, pr, d] * rs[:, pr]
        src = bass.AP(ps_y.tensor, ps_y.offset,
                      [list(ps_y.ap[0]), [1, HD], [65, GP]])
        nc.vector.tensor_mul(
            ystc3[:, :, g * GP:(g + 1) * GP],
            src,
            _bcast(rs[:, 0:1], [[0, HD], [1, GP]]),
        )

    prev = None
    ngrp = THALF // GP
    qq = ngrp // 8
    for g in range(ngrp):
        cur = scores(g)
        if prev is not None:
            finish(*prev)
        prev = cur
        # finish(g-1) has run; eighth q is complete once finish(q*qq+qq-1)
        # has been emitted, i.e. at g == (q+1)*qq
        if g % qq == 0 and g > 0:
            y_flush(g // qq - 1)
    finish(*prev)
    y_flush(7)


def _emit_stage3(ctx, tc, R, Wpl, b_prj_sb, wpool, ps1, yt_sb, outT, wp_pre):
    nc = tc.nc
    outp = ctx.enter_context(tc.tile_pool(name=R + "outp", bufs=3))
    for mo in range(NK):
        if mo in wp_pre:
            wp = wp_pre[mo]
        else:
            wp = wpool.tile([128, NK * 128], BF16, name=f"{R}wp{mo}", tag="wa")
            nc.sync.dma_start(wp[:], Wpl.ap()[mo * 128:(mo + 1) * 128, :])
        ps = ps1.tile([128, NTOK], F32, name=f"{R}pso{mo}", tag="ps")
        for kc in range(NK):
            nc.tensor.matmul(
                ps[:],
                wp[:, kc * 128:(kc + 1) * 128],
                yt_sb[:, kc * NTOK:(kc + 1) * NTOK],
                start=(kc == 0),
                stop=(kc == NK - 1),
            )
        ob = outp.tile([128, NTOK], F32, name=f"{R}ob{mo}", tag="ob")
        bb = b_prj_sb[:, mo:mo + 1]
        nc.vector.tensor_add(ob[:], ps[:], _bcast(bb, [[0, NTOK]]))
        nc.scalar.dma_start(outT.ap()[mo * 128:(mo + 1) * 128, :], ob[:])


_PROGRAMS = {}


def _get_program(reps=1):
    if reps not in _PROGRAMS:
        _PROGRAMS[reps] = _build_program(reps)
    return _PROGRAMS[reps]


def _host_inputs(x, W_atten, b_atten, W_proj, b_proj):
    bf = ml_dtypes.bfloat16
    x = np.asarray(x, np.float32).reshape(B, T, C)
    Wa = np.asarray(W_atten, np.float32).copy()
    ba = np.asarray(b_atten, np.float32).copy()
    Wa[C:2 * C] *= 0.125  # fold the 1/sqrt(hd) score scale into K
    ba[C:2 * C] *= 0.125
    # Wal[m*128+p, k*128+o] = Wa[m*128+o, k*128+p] (contiguous 1MB tiles)
    Wal = np.ascontiguousarray(
        Wa.reshape(NM, 128, NK, 128).transpose(0, 3, 2, 1)
    ).astype(bf).reshape(NM * 128, NK * 128)
    Wpl = np.ascontiguousarray(
        np.asarray(W_proj, np.float32).reshape(NK, 128, NK, 128)
        .transpose(0, 3, 2, 1)
    ).astype(bf).reshape(NK * 128, NK * 128)
    b_att_h = np.ascontiguousarray(ba.reshape(NM, 128).T)
    b_prj_h = np.ascontiguousarray(
        np.asarray(b_proj, np.float32).reshape(NK, 128).T
    )
    mask = np.tril(np.ones((H, H), np.float32))  # keep j<=i
    maskT2_h = np.ascontiguousarray(np.vstack([mask.T, mask.T])).astype(bf)

    shard_b = B // N_CORES
    in_maps = []
    for i in range(N_CORES):
        xs = x[i * shard_b:(i + 1) * shard_b].reshape(NTOK, C)
        xl_i = np.ascontiguousarray(
            xs.reshape(NTOK, NK, 128).transpose(2, 1, 0)
        ).astype(bf).reshape(128, NK * NTOK)
        in_maps.append(
            {
                "xl": xl_i,
                "Wal": Wal,
                "Wpl": Wpl,
                "b_att": b_att_h,
                "b_prj": b_prj_h,
                "maskT2": maskT2_h,
            }
        )
    return in_maps


def _host_outputs(results):
    shard_b = B // N_CORES
    out = np.empty((B, T, C), dtype=np.float32)
    for i in range(N_CORES):
        oT = np.asarray(results[i]["outT"], np.float32)  # (C, NTOK)
        out[i * shard_b:(i + 1) * shard_b] = oT.T.reshape(shard_b, T, C)
    return out


def run(inputs, trace=False):
    nc = _get_program()
    in_maps = _host_inputs(**inputs)
    res = run_bass_kernel_spmd(
        nc, in_maps, list(range(N_CORES)), trace=trace
    )
    return _host_outputs(res.results), res


def kernel(x, W_atten, b_atten, W_proj, b_proj):
    out, _ = run(
        dict(
            x=x,
            W_atten=W_atten,
            b_atten=b_atten,
            W_proj=W_proj,
            b_proj=b_proj,
        )
    )
    return out


# revision 25
# speedup vs baseline: 12.4208x; 1.0172x over previous
"""nn_Attention_30511447671564 — Trainium2 Bass kernel (v2).

Head-mixing attention block. Shapes (hardcoded): B=64, T=64, C=4096,
H=64, hd=64, rank=1.  For every token (b,t): attention mixes the 64
heads (HxH scores, causal over head index).

    qkv = x @ W_atten^T + b_atten                  (B,T,3C)
    per-token: s[i,j] = q_i . k_j / 8  (i,j heads, causal j<=i)
               att = softmax_j(s);  y_i = sum_j att[i,j] v_j
    out = y @ W_proj^T + b_proj                    (B,T,C)

Distribution: pure data-parallel — 8 cores x 512 tokens, no collectives.

v2 changes vs v1 (trace-driven; v1 was DMA-bound at 146 GB/s with 270B
packets and a serialized DMA queue):
  * Weights host-prepacked so each 128x(32*128) stationary tile is one
    contiguous 1MB DRAM read (8KB per partition) — one DMA per tile.
  * Stage 1 in a single 512-token pass (halves weight traffic vs two
    256-token segments).
  * The 1/8 score scale is folded into the K rows of W_atten/b_atten on
    the host; PSUM evictions are bias-adds on the (idle) Vector engine.
  * DMAs split across two hardware queues: loads on the Sync queue,
    bounce-buffer writes on the Scalar queue (queues serialize per
    engine; v1 put everything on Sync).
  * Stage 1 emits K features first, then Q, then V, so the attention
    stage's gather DMAs overlap the tail of stage 1.
  * Attention: tokens t and t+256 are processed as a pair in opposite
    PE-array quadrants (tile_position (0,0)/(64,64)) — q/k/v for the
    two half-chunks are stacked on partitions 0:64 / 64:128.  The
    softmax denominator comes free as a 65th "ones" column of v
    (killing the per-token denominator matmul + LDWEIGHTS of v1).
"""

import numpy as np
import ml_dtypes
from contextlib import ExitStack

import concourse.bass as bass
import concourse.tile as tile
from concourse import bacc, mybir
from concourse.bass_utils import run_bass_kernel_spmd

F32 = mybir.dt.float32
BF16 = mybir.dt.bfloat16
ACT = mybir.ActivationFunctionType

N_CORES = 8
B, T, C = 64, 64, 4096
H, HD = 64, 64
NTOK = (B // N_CORES) * T            # 512 tokens per core
O3 = 3 * C                           # 12288
NM = O3 // 128                       # 96 qkv feature tiles of 128
NK = C // 128                        # 32 contraction tiles of 128
THALF = NTOK // 2                    # 256: attention half-chunk length
GP = 4                               # token-pairs per attention group


def _bcast(ap, dims):
    """AP broadcast helper: keep partition dim, append given free dims."""
    return bass.AP(ap.tensor, ap.offset, [list(ap.ap[0])] + dims)


def _build_program(reps=1, stages=(1, 2, 3)):
    nc = bacc.Bacc(
        "TRN2", target_bir_lowering=False, debug=False, num_devices=N_CORES
    )

    xl = nc.declare_dram_parameter("xl", [128, NK * NTOK], BF16, isOutput=False)
    Wal = nc.declare_dram_parameter("Wal", [NM * 128, NK * 128], BF16, isOutput=False)
    Wpl = nc.declare_dram_parameter("Wpl", [NK * 128, NK * 128], BF16, isOutput=False)
    b_att = nc.declare_dram_parameter("b_att", [128, NM], F32, isOutput=False)
    b_prj = nc.declare_dram_parameter("b_prj", [128, NK], F32, isOutput=False)
    maskT2 = nc.declare_dram_parameter("maskT2", [128, H], BF16, isOutput=False)
    outT = nc.declare_dram_parameter("outT", [C, NTOK], F32, isOutput=True)

    with tile.TileContext(nc) as tc, ExitStack() as ctx:
        for rep in range(reps):
            with ExitStack() as rctx:
                _emit(rctx, tc, xl, Wal, Wpl, b_att, b_prj, maskT2, outT,
                      rep, stages)
    nc.compile()
    return nc


def _emit(ctx, tc, xl, Wal, Wpl, b_att, b_prj, maskT2, outT, rep=0,
          stages=(1, 2, 3)):
    nc = tc.nc
    R = f"r{rep}_"

    const = ctx.enter_context(tc.tile_pool(name=R + "const", bufs=1))
    b_att_sb = const.tile([128, NM], F32)
    nc.sync.dma_start(b_att_sb[:], b_att.ap())
    b_prj_sb = const.tile([128, NK], F32)
    nc.sync.dma_start(b_prj_sb[:], b_prj.ap())
    mask_sb = const.tile([128, H], BF16)
    nc.sync.dma_start(mask_sb[:], maskT2.ap())
    # prewarm the scalar engine's Exp table set (~2.7us) before stage 2
    warm = const.tile([1, 1], F32)
    nc.scalar.activation(warm[:], mask_sb[0:1, 0:1], ACT.Exp)

    dram = ctx.enter_context(tc.tile_pool(name=R + "dram", bufs=1, space="DRAM"))
    Qrot = dram.tile([HD, H * NTOK], BF16)    # [d, (h, t)]
    Krot = dram.tile([HD, H * NTOK], BF16)    # [d, (h, t)]
    Vrot = dram.tile([H, HD * NTOK], BF16)    # [h, (d, t)]
    Ydram = dram.tile([H, HD * NTOK], BF16)   # [i, (d, t)]

    q3 = Qrot.rearrange("d (h t) -> d h t", t=NTOK)
    k3 = Krot.rearrange("d (h t) -> d h t", t=NTOK)
    v3 = Vrot.rearrange("h (d t) -> h d t", t=NTOK)
    y3 = Ydram.rearrange("i (d t) -> i d t", t=NTOK)

    wpool = ctx.enter_context(tc.tile_pool(name=R + "wpool", bufs=4))
    ps1 = ctx.enter_context(tc.tile_pool(name=R + "ps1", bufs=2, space="PSUM"))

    # attention-stage tiles exist up front so stage 1 can gather into them
    qkp = ctx.enter_context(tc.tile_pool(name=R + "qkp", bufs=1))
    qp = qkp.tile([128, H * THALF], BF16)     # [(p2,d), (i, t')]
    kp = qkp.tile([128, H * THALF], BF16)
    v4 = qkp.tile([128, 65 * THALF], BF16)    # [(p2,j), (d|ones, t')]
    ystc = qkp.tile([128, HD * THALF], BF16)  # [(p2,i), (d, t')]
    qpv = qp.rearrange("p (i t) -> p i t", t=THALF)
    kpv = kp.rearrange("p (i t) -> p i t", t=THALF)
    v4v = v4.rearrange("p (d t) -> p d t", t=THALF)
    ystc3 = ystc.rearrange("p (d t) -> p d t", t=THALF)
    nc.vector.memset(v4v[:, 64, :], 1.0)      # fused-denominator ones row

    def gathers(c):
        """Gather 8 heads' worth of q/k/v produced by the last 4 stage-1
        tiles into the stacked attention layouts.  q/k chunks write
        across 64 partitions (fast, Sync queue); v chunks concentrate
        32KB on 8 partitions (partition-write-bound, ~12us) so they go
        on the Scalar queue in 4-head pieces to not starve the weight
        loads."""
        if c < 8:
            h0, src, dstv = 8 * c, k3, kpv
        elif c < 16:
            # DMA duration scales with per-partition bytes (32KB per head
            # partition here), so chunking by heads only serializes it:
            # issue the v gather as 4 DMAs spanning all 64 head partitions
            # (d-halves, ~6us each, Scalar queue).  Must come after ALL
            # v rotation writes (program-order read-before-write is
            # undefined data), i.e. only at c == 15.
            if c == 15:
                for dd in (0, 32):
                    nc.scalar.dma_start(
                        v4v[0:64, dd:dd + 32, :], v3[:, dd:dd + 32, 0:THALF]
                    )
                    nc.scalar.dma_start(
                        v4v[64:128, dd:dd + 32, :],
                        v3[:, dd:dd + 32, THALF:NTOK],
                    )
            return
        else:
            h0 = 8 * (c - 16)
            if c == 23:
                return  # final heads gathered per-tile (gathers_tail)
            h0, src, dstv = h0, q3, qpv
        nc.sync.dma_start(dstv[0:64, h0:h0 + 8, :], src[:, h0:h0 + 8, 0:THALF])
        nc.sync.dma_start(
            dstv[64:128, h0:h0 + 8, :], src[:, h0:h0 + 8, THALF:NTOK]
        )

    def gathers_tail(h0):
        """Per-tile q gathers for the last 4 Q tiles (2 heads each) so
        the attention start only waits on the final tile's rotation."""
        nc.sync.dma_start(qpv[0:64, h0:h0 + 2, :], q3[:, h0:h0 + 2, 0:THALF])
        nc.sync.dma_start(
            qpv[64:128, h0:h0 + 2, :], q3[:, h0:h0 + 2, THALF:NTOK]
        )

    if 1 in stages:
        _emit_stage1(ctx, tc, R, xl, Wal, b_att_sb, wpool, ps1, q3, k3, v3,
                     gathers, gathers_tail)

    # prefetch the first two projection weight tiles during attention
    wp_pre = {}
    for mo in range(2):
        wp = wpool.tile([128, NK * 128], BF16, name=f"{R}wp{mo}", tag="wa")
        nc.sync.dma_start(wp[:], Wpl.ap()[mo * 128:(mo + 1) * 128, :])
        wp_pre[mo] = wp

    ypool = ctx.enter_context(tc.tile_pool(name=R + "ypool", bufs=1))
    yt_sb = ypool.tile([128, NK * NTOK], BF16)
    yv = yt_sb.rearrange("(p2 d) (k t) -> p2 d k t", p2=2, t=NTOK)
    dv = Ydram.rearrange("(k p2) (d t) -> p2 d k t", p2=2, t=NTOK)

    def y_write(a, b):
        """Write attention-output t' slice [a,b) to the DRAM bounce
        (Scalar queue, between exp ops)."""
        nc.scalar.dma_start(y3[:, :, a:b], ystc3[0:64, :, a:b])
        nc.scalar.dma_start(
            y3[:, :, THALF + a:THALF + b], ystc3[64:128, :, a:b]
        )

    def y_load(a, b):
        """Load bounce t' slice [a,b) back in stage-3 layout (Sync)."""
        for p2 in range(2):
            nc.sync.dma_start(yv[p2][:, :, a:b], dv[p2][:, :, a:b])
            nc.sync.dma_start(
                yv[p2][:, :, THALF + a:THALF + b],
                dv[p2][:, :, THALF + a:THALF + b],
            )

    if 2 in stages:
        _emit_stage2(ctx, tc, R, mask_sb, qpv, kpv, v4v, ystc3,
                     y_write, y_load)
    if 3 in stages:
        _emit_stage3(ctx, tc, R, Wpl, b_prj_sb, wpool, ps1, yt_sb, outT,
                     wp_pre)


def _emit_stage1(ctx, tc, R, xl, Wal, b_att_sb, wpool, ps1, q3, k3, v3,
                 gathers, gathers_tail):
    nc = tc.nc
    with tc.tile_pool(name=R + "xpool", bufs=1) as xpool, \
         tc.tile_pool(name=R + "secp", bufs=8) as secp:
        x_sb = xpool.tile([128, NK * NTOK], BF16)
        # first x chunk ahead of the first weight tile; rest behind it
        XSP = 8
        nc.sync.dma_start(
            x_sb[:, 0:XSP * NTOK], xl.ap()[:, 0:XSP * NTOK]
        )

        # K, V, Q tile order; after every 4 tiles the 8 heads they
        # produced are gathered into the attention-stage layouts (the
        # gather DMAs interleave with weight loads on the Sync queue).
        morder = list(range(NK, 2 * NK)) + list(range(2 * NK, NM)) + \
            list(range(NK))
        for mi, m in enumerate(morder):
            wa = wpool.tile([128, NK * 128], BF16, name=f"{R}wa{m}", tag="wa")
            nc.sync.dma_start(wa[:], Wal.ap()[m * 128:(m + 1) * 128, :])
            if mi == 0:
                for xc in range(XSP, NK, XSP):
                    nc.sync.dma_start(
                        x_sb[:, xc * NTOK:(xc + XSP) * NTOK],
                        xl.ap()[:, xc * NTOK:(xc + XSP) * NTOK],
                    )
            ps = ps1.tile([128, NTOK], F32, name=f"{R}ps{m}", tag="ps")
            for kc in range(NK):
                nc.tensor.matmul(
                    ps[:],
                    wa[:, kc * 128:(kc + 1) * 128],
                    x_sb[:, kc * NTOK:(kc + 1) * NTOK],
                    start=(kc == 0),
                    stop=(kc == NK - 1),
                )
            sec = secp.tile([128, NTOK], BF16, name=f"{R}sec{m}", tag="sec")
            bb = b_att_sb[:, m:m + 1]
            nc.vector.tensor_add(sec[:], ps[:], _bcast(bb, [[0, NTOK]]))
            # rotate the two heads of this tile out to the DRAM bounce
            if m < NK:
                dst3, h0 = q3, 2 * m
            elif m < 2 * NK:
                dst3, h0 = k3, 2 * (m - NK)
            else:
                dst3, h0 = None, 2 * (m - 2 * NK)
            for h2 in range(2):
                s2 = sec[h2 * 64:h2 * 64 + 64, :]
                if dst3 is not None:
                    nc.scalar.dma_start(dst3[:, h0 + h2, :], s2)
                else:
                    nc.scalar.dma_start(v3[h0 + h2, :, :], s2)
            if mi >= 92:
                gathers_tail(2 * m)
            if mi % 4 == 3:
                gathers(mi // 4)


def _emit_stage2(ctx, tc, R, mask_sb, qpv, kpv, v4v, ystc3, y_write, y_load):
    nc = tc.nc
    psA = ctx.enter_context(tc.tile_pool(name=R + "psA", bufs=3, space="PSUM"))
    psB = ctx.enter_context(tc.tile_pool(name=R + "psB", bufs=3, space="PSUM"))
    atp = ctx.enter_context(tc.tile_pool(name=R + "atp", bufs=5))

    def scores(g):
        ps_s = psA.tile([128, GP * H], F32, name=f"{R}pss{g}", tag="pss")
        for pl in range(GP):
            t = g * GP + pl
            nc.tensor.matmul(
                ps_s[0:64, pl * H:(pl + 1) * H],
                kpv[0:64, :, t], qpv[0:64, :, t],
                start=True, stop=True, tile_position=(0, 0),
            )
            nc.tensor.matmul(
                ps_s[64:128, pl * H:(pl + 1) * H],
                kpv[64:128, :, t], qpv[64:128, :, t],
                start=True, stop=True, tile_position=(64, 64),
            )
        exp_sb = atp.tile([128, GP * H], BF16, name=f"{R}exp{g}", tag="exp")
        nc.scalar.activation(exp_sb[:], ps_s[:], ACT.Exp)
        att = atp.tile([128, GP * H], BF16, name=f"{R}att{g}", tag="att")
        nc.vector.tensor_mul(
            att[:], exp_sb[:], _bcast(mask_sb[:, 0:1], [[0, GP], [1, H]])
        )
        return g, att

    def finish(g, att):
        ps_y = psB.tile([128, GP * 65], F32, name=f"{R}psy{g}", tag="psy")
        py3 = ps_y.rearrange("p (pr e) -> p pr e", e=65)
        for pl in range(GP):
            t = g * GP + pl
            a0 = att[0:64, pl * H:(pl + 1) * H]
            a1 = att[64:128, pl * H:(pl + 1) * H]
            nc.tensor.matmul(
                py3[0:64, pl, :], a0, v4v[0:64, :, t],
                start=True, stop=True, tile_position=(0, 0),
            )
            nc.tensor.matmul(
                py3[64:128, pl, :], a1, v4v[64:128, :, t],
                start=True, stop=True, tile_position=(64, 64),
            )
        rs = atp.tile([128, GP], F32, name=f"{R}rs{g}", tag="rs")
        nc.vector.reciprocal(rs[:], py3[:, :, 64])
        # ystc[:, d, g*GP + pr] = ps_y[:, pr, d] * rs[:, pr]
        src = bass.AP(ps_y.tensor, ps_y.offset,
                      [list(ps_y.ap[0]), [1, HD], [65, GP]])
        nc.vector.tensor_mul(
            ystc3[:, :, g * GP:(g + 1) * GP],
            src,
            _bcast(rs[:, 0:1], [[0, HD], [1, GP]]),
        )

    # depth-2 software pipeline: scores run two groups ahead of finish
    ngrp = THALF // GP
    qq = ngrp // 8          # groups per eighth (t' slice of 32)
    pend = [scores(0), scores(1)]
    for g in range(2, ngrp + 2):
        if g < ngrp:
            pend.append(scores(g))
        finish(*pend.pop(0))
        gf = g - 2          # finished group index
        # eighth q complete after finish(q*qq + qq - 1)
        if (gf + 1) % qq == 0:
            q = (gf + 1) // qq - 1
            y_write(q * 32, (q + 1) * 32)
            # loads: quarters 0-2 lag the writes; final quarter as eighths
            if q in (1, 3, 5):
                a = (q // 2) * 64
                y_load(a, a + 64)
            elif q >= 6:
                y_load(q * 32, (q + 1) * 32)


def _emit_stage3(ctx, tc, R, Wpl, b_prj_sb, wpool, ps1, yt_sb, outT, wp_pre):
    nc = tc.nc
    outp = ctx.enter_context(tc.tile_pool(name=R + "outp", bufs=3))
    for mo in range(NK):
        if mo in wp_pre:
            wp = wp_pre[mo]
        else:
            wp = wpool.tile([128, NK * 128], BF16, name=f"{R}wp{mo}", tag="wa")
            nc.sync.dma_start(wp[:], Wpl.ap()[mo * 128:(mo + 1) * 128, :])
        ps = ps1.tile([128, NTOK], F32, name=f"{R}pso{mo}", tag="ps")
        for kc in range(NK):
            nc.tensor.matmul(
                ps[:],
                wp[:, kc * 128:(kc + 1) * 128],
                yt_sb[:, kc * NTOK:(kc + 1) * NTOK],
                start=(kc == 0),
                stop=(kc == NK - 1),
            )
        ob = outp.tile([128, NTOK], F32, name=f"{R}ob{mo}", tag="ob")
        bb = b_prj_sb[:, mo:mo + 1]
        nc.vector.tensor_add(ob[:], ps[:], _bcast(bb, [[0, NTOK]]))
        nc.scalar.dma_start(outT.ap()[mo * 128:(mo + 1) * 128, :], ob[:])


_PROGRAMS = {}


def _get_program(reps=1):
    if reps not in _PROGRAMS:
        _PROGRAMS[reps] = _build_program(reps)
    return _PROGRAMS[reps]


def _host_inputs(x, W_atten, b_atten, W_proj, b_proj):
    bf = ml_dtypes.bfloat16
    x = np.asarray(x, np.float32).reshape(B, T, C)
    Wa = np.asarray(W_atten, np.float32).copy()
    ba = np.asarray(b_atten, np.float32).copy()
    Wa[C:2 * C] *= 0.125  # fold the 1/sqrt(hd) score scale into K
    ba[C:2 * C] *= 0.125
    # Wal[m*128+p, k*128+o] = Wa[m*128+o, k*128+p] (contiguous 1MB tiles)
    Wal = np.ascontiguousarray(
        Wa.reshape(NM, 128, NK, 128).transpose(0, 3, 2, 1)
    ).astype(bf).reshape(NM * 128, NK * 128)
    Wpl = np.ascontiguousarray(
        np.asarray(W_proj, np.float32).reshape(NK, 128, NK, 128)
        .transpose(0, 3, 2, 1)
    ).astype(bf).reshape(NK * 128, NK * 128)
    b_att_h = np.ascontiguousarray(ba.reshape(NM, 128).T)
    b_prj_h = np.ascontiguousarray(
        np.asarray(b_proj, np.float32).reshape(NK, 128).T
    )
    mask = np.tril(np.ones((H, H), np.float32))  # keep j<=i
    maskT2_h = np.ascontiguousarray(np.vstack([mask.T, mask.T])).astype(bf)

    shard_b = B // N_CORES
    in_maps = []
    for i in range(N_CORES):
        xs = x[i * shard_b:(i + 1) * shard_b].reshape(NTOK, C)
        xl_i = np.ascontiguousarray(
            xs.reshape(NTOK, NK, 128).transpose(2, 1, 0)
        ).astype(bf).reshape(128, NK * NTOK)
        in_maps.append(
            {
                "xl": xl_i,
                "Wal": Wal,
                "Wpl": Wpl,
                "b_att": b_att_h,
                "b_prj": b_prj_h,
                "maskT2": maskT2_h,
            }
        )
    return in_maps


def _host_outputs(results):
    shard_b = B // N_CORES
    out = np.empty((B, T, C), dtype=np.float32)
    for i in range(N_CORES):
        oT = np.asarray(results[i]["outT"], np.float32)  # (C, NTOK)
        out[i * shard_b:(i + 1) * shard_b] = oT.T.reshape(shard_b, T, C)
    return out


def run(inputs, trace=False):
    nc = _get_program()
    in_maps = _host_inputs(**inputs)
    res = run_bass_kernel_spmd(
        nc, in_maps, list(range(N_CORES)), trace=trace
    )
    return _host_outputs(res.results), res


def kernel(x, W_atten, b_atten, W_proj, b_proj):
    out, _ = run(
        dict(
            x=x,
            W_atten=W_atten,
            b_atten=b_atten,
            W_proj=W_proj,
            b_proj=b_proj,
        )
    )
    return out
